# revision 50
# baseline (speedup 1.0000x reference)
"""Trainium2 Bass kernel for NanoAttention (B=4, T=2048, C=1024, H=16, causal).

Sharding: 8 cores = 4 batches x 2 head-groups (8 heads each).
Per core (b, hg):
  - column-parallel qkv:  q,k produced in [channel, token] (transposed) layout,
    v produced in [token, channel] (natural) layout with an appended ones
    column per head (fuses the softmax denominator into the AV matmul).
  - causal attention per head with S^T in [key, query] layout; exp on ACT with
    scale=1/sqrt(D); block-causal skipping + triangular masking on the
    diagonal blocks.
  - row-parallel proj producing a partial output [C, T] in bf16; the host adds
    the two head-group partials per batch and transposes back.

Session-2 optimizations (271us -> ~256us min / ~257us median):
  - k projection in fp8e4 DoubleRow: W_k host-prescaled by 64 (fp8e4
    normal range; 1/64 folded into the exp scale) and x in fp8; each k
    chain is 4 matmuls of 256-contraction instead of 8 of 128 (half the
    PE time; DoubleRow measured at the SAME ns/instr as bf16 = 2x MACs,
    NOT the cost model's 0.5 cyc/row). fp8 on BOTH q and k sims to
    2.0e-2 rel err (at the gate); k-only gives 1.2e-2 measured.
  - one shared 3-slot psum pool (6 banks) for S blocks + fill chains
    (deeper rotation absorbs ACT/DVE latency spikes).
  - per-hp normalize split: denominators copied psum->sbuf FIRST (2 tiny
    copies, so sums is ready early), then Ln/Exp/broadcast/muls deferred
    into the NEXT hp's pair loop -- the in-order ACT queue used to
    suspend ~2-3us at the Ln waiting on DVE-queued sums, blocking ready
    attention exps and stalling the PE on psum-slot reuse.
  - diag tri-masks cover both heads in one DVE op (tri broadcast over r).
  - vp tiles pre-allocated at startup (ones column memset in idle time,
    zero-pad dropped: garbage cols 65:127 feed psum partitions nobody
    reads).
  - the last two output stores are unpaired so the final DMA launches as
    soon as its own chain lands.
  Failed experiments (recorded so they are not retried): DVE
  InstReciprocal = 3.3us/[33,512] (queue-blocking; Ln+Exp on ACT is far
  better, and ACT-table Reciprocal is in a different table set than Exp
  so it would pay 1.3us table switches); dma_start on the ACT HWDGE
  queue corrupts input data in this container; a single static psum tile
  with python-side slot rotation (for paired exps over adjacent slots)
  serializes the kernel to ~490us (subview dependency tracking is
  tile-granular); exp pairing via pool tiles is psum-capacity-blocked;
  hi/lo fp8 splits lose (3 terms x 2x-MACs > bf16); fill rebalancing
  (v(3) into attn2 etc.) only relocates the starvation.

Optimizations vs the 319us v1 baseline (session 1, ~271us, ~15%):
  - software-pipelined emission with a stride-2 rotation: the PE queue is
    in-order, so [S(b), S(b+1)] [dense fill] [AV(b-2), AV(b-1)] keeps the
    AVs off the exp critical path and the pair batching halves the ~130ns
    per-insertion PE restart cost.
  - dense work (qkv of the next group, v chains, ALL proj chains) is
    drip-fed between attention blocks in whole-chain granularity; proj is
    deferred to attn3, which has by far the most ACT(exp)-paced attention
    to fill; 5 chains are reserved for the kernel tail so the PE (and the
    HAM clock gate) stays busy through the final normalize.
  - every dma_start costs ~630ns of serial descriptor generation on the
    sync engine: inputs/outputs are host-pre-tiled ([128, n, free] with
    >=2KB contiguous per-partition runs) and loaded/stored with few
    multi-chunk transfers; output stores are paired.
  - PE pre-warm: ~15 dummy matmuls gated only on a memset lift the HAM
    clock gate from 4/8 (1.2GHz) to 8/8 (2.4GHz) before the first real
    matmul's DMA dependencies can possibly arrive (~13us).
  - vp stationaries padded to 128 columns (v | ones | zeros): full-width
    weights qualify for FWL so AV weight loads hide under the previous
    matmul (65-wide loads cannot and cost ~+120ns per AV pair).
  - the softmax reciprocal broadcast goes SBUF->DRAM->broadcast-DMA, and
    the yt normalize multiplies run on gpsimd so the roundtrip latency
    never head-of-line-blocks the in-order DVE queue (which gates dense
    chain psum reuse and the causal tri masks); the kernel-tail broadcast
    uses a PE ones-matmul directly from rec with the multiply reading
    PSUM, skipping two copies.
  - output DMA in bf16 (host accumulates the two head-group partials in
    f32; rel err unaffected at 4.3e-3 vs the 2e-2 gate).
  (gpsimd partition_broadcast and the custom-DVE reciprocal_approx_fast
  do not compile in this container's walrus: "ISA wrong length".)
"""
import os
import sys

sys.path.insert(0, '/opt/trn_rl_repo')

import numpy as np
import orjson

import concourse.bass as bass
import concourse.mybir as mybir
import concourse.tile as tile
from concourse.bass_utils import run_bass_kernel_spmd

# ---------------------------------------------------------------------------
# Workaround for this container's walrus build: it enforces the HW limit of
# one sync-wait per instruction (two for EventSemaphore), but Tile's sem
# assignment can emit more (kernel-tail Drain waits on every DMA queue used;
# HWDGE stores can pick up two queue waits). Split the overflow onto
# preceding pure-wait EventSemaphore instructions on the same engine at
# JSON-serialization time so every compile path is covered.
# ---------------------------------------------------------------------------


def _split_multi_waits(data):
    n_split = 0
    for func in data.get("functions", []):
        for blk in func.get("blocks", []):
            insts = blk.get("instructions")
            if not insts:
                continue
            out = []
            for inst in insts:
                si = inst.get("sync_info")
                waits = (si or {}).get("on_wait") or []
                cap = 2 if inst.get("opcode") == "EventSemaphore" else 1
                if len(waits) > cap and "engine" in inst:
                    extra = waits[:-cap]
                    si["on_wait"] = waits[-cap:]
                    for i in range(0, len(extra), 2):
                        n_split += 1
                        out.append({
                            "debug": inst.get("debug"),
                            "engine": inst["engine"],
                            "ins": [],
                            "outs": [],
                            "name": f"{inst['name']}_wsplit{n_split}",
                            "opcode": "EventSemaphore",
                            "sync_info": {"on_wait": extra[i:i + 2],
                                          "on_update": []},
                        })
                out.append(inst)
            blk["instructions"] = out
    return data


_orig_to_json_bytes = bass.Bass.to_json_bytes


def _patched_to_json_bytes(self):
    return orjson.dumps(_split_multi_waits(orjson.loads(_orig_to_json_bytes(self))))


bass.Bass.to_json_bytes = _patched_to_json_bytes

# ---------------------------------------------------------------------------

B, T, C = 4, 2048, 1024
N_HEAD, D = 16, 64
HLOC = 8          # heads per core
CLOC = HLOC * D   # 512 local qkv channels per core
QG = 512          # query-group width
NG = T // QG      # 4 query groups
KB = 128          # key-block width
F32R = mybir.dt.float32r
F32 = mybir.dt.float32
BF16 = mybir.dt.bfloat16
CDT = BF16
ADT = CDT
FP8 = mybir.dt.float8e4
DR = mybir.MatmulPerfMode.DoubleRow
KSC = 64.0        # host pre-scale on W_k so fp8e4 stays in normal range
EXP = mybir.ActivationFunctionType.Exp
SCALE = 1.0 / np.sqrt(D) / KSC   # folds the k fp8 pre-scale out of S
# reciprocal broadcast path: "dma" (DRAM roundtrip broadcast) or "gpsimd"
# (partition_broadcast — does NOT compile in this container's walrus:
# "ISA wrong length", same for the custom-DVE reciprocal_approx_fast).
BCAST = os.environ.get("ATTN_BCAST", "dma")
# engine for the yt normalize multiplies: "gpsimd" keeps the DMA-broadcast
# wait out of the DVE queue; "dve" is the fallback.
MUL = os.environ.get("ATTN_MUL", "gpsimd")


def _build_body(nc, tc, ctx, xt, x8d, wqkt, wk8d, wvt, wpt, tri, ot):
    p_wqk = ctx.enter_context(tc.tile_pool(name="wqk", bufs=4))
    p_wk8 = ctx.enter_context(tc.tile_pool(name="wk8", bufs=1))
    p_x8 = ctx.enter_context(tc.tile_pool(name="x8", bufs=2))
    p_wv = ctx.enter_context(tc.tile_pool(name="wv", bufs=1))
    p_wp = ctx.enter_context(tc.tile_pool(name="wp", bufs=1))
    p_xt0 = ctx.enter_context(tc.tile_pool(name="xt0", bufs=4))
    p_xt = ctx.enter_context(tc.tile_pool(name="xt", bufs=2))
    p_k = ctx.enter_context(tc.tile_pool(name="ksb", bufs=4))
    p_q = ctx.enter_context(tc.tile_pool(name="qsb", bufs=8))
    p_vp = ctx.enter_context(tc.tile_pool(name="vp", bufs=16))
    p_es = ctx.enter_context(tc.tile_pool(name="es", bufs=7))
    # all four groups' yt stay alive: proj is deferred to attn3/tail
    p_yt = ctx.enter_context(tc.tile_pool(name="yt", bufs=16))
    p_ost = ctx.enter_context(tc.tile_pool(name="ost", bufs=4))
    p_one = ctx.enter_context(tc.tile_pool(name="one", bufs=1))
    p_rec = ctx.enter_context(tc.tile_pool(name="rec", bufs=3))
    p_ysb = ctx.enter_context(tc.tile_pool(name="ysb", bufs=4))
    p_bc = ctx.enter_context(tc.tile_pool(name="bc", bufs=2))
    if BCAST == "dma":
        p_drb = ctx.enter_context(tc.tile_pool(name="drb", bufs=2, space="DRAM"))
    # ONE shared 3-slot psum pool (6 banks) for S blocks AND dense-fill
    # chains: the extra rotation depth gives the S exp (and fill copies)
    # ~2-3us of slack before their slot is reused, absorbing ACT/DVE queue
    # latency spikes that used to stall the in-order PE queue ~1.4us at a
    # time with the old 2-buf ps_s.
    # NOTE: a single static [128,3,2,QG] psum tile with python-side slot
    # rotation (to enable paired exps over adjacent slots) serializes the
    # whole kernel to ~490us: the tile framework's dependency tracking on
    # subviews of one tile is far more conservative than across pool tiles.
    # Keep pool-based rotation.
    ps_sh = ctx.enter_context(tc.tile_pool(name="pssh", bufs=3, space="PSUM"))
    ps_y = ctx.enter_context(tc.tile_pool(name="psy", bufs=2, space="PSUM"))

    def sh_tile(name):
        return ps_sh.tile([128, 2, QG], F32, tag="pss", name=name)

    # ---- static state ----
    # Each dma_start costs ~630ns of serial descriptor-generation on the
    # sync engine, so inputs are loaded with FEW multi-chunk transfers:
    # contraction-chunk views are AP slices of [128, n, free] tiles.
    wqk_sb = [None] * 8   # views: wqk_sb[kc] = [128, CLOC] (q-only, bf16)
    wv_sb = [None] * 8
    wp_sb = []
    xt_g = [[None] * 8 for _ in range(NG)]
    x8_g = [None] * NG    # [128, 8, QG] fp8 tiles (k-chain moving operand)
    wk8_sb = None         # [128, 4, 2, CLOC] fp8 (k DoubleRow stationaries)
    tri_sb = p_one.tile([KB, KB], CDT, tag="tri")
    ones_sb = p_one.tile([33, 64], F32R, tag="ones")
    nc.vector.memset(ones_sb.bitcast(F32), 1.0)
    k_sb = [p_k.tile([128, T], ADT, tag="ksb", name=f"ksb{c}") for c in range(4)]
    q_gs = [[None] * 4 for _ in range(NG)]
    vp_sb = []        # grows to 16 tiles, 4 per group
    yt_gs = [None] * NG

    def dma_x(g, kcs, pool, eng=None):
        # one dma_start covering contraction chunks kcs (list); the host
        # pre-tiles x as [NG, 128, 8, QG] so each partition's run is
        # contiguous (n*1KB descriptors instead of n*128)
        n = len(kcs)
        t = pool.tile([128, n, QG], CDT, tag="xt", name=f"xt{g}_{kcs[0]}")
        (eng or nc.sync).dma_start(out=t, in_=xt[g, :, kcs[0]:kcs[0] + n, :])
        for i, kc in enumerate(kcs):
            xt_g[g][kc] = t[:, i, :]

    def dma_x8(g):
        t = p_x8.tile([128, 8, QG], FP8, tag="x8", name=f"x8_{g}")
        nc.sync.dma_start(out=t, in_=x8d[g, :, :, :])
        x8_g[g] = t

    def dma_wqk(kc0):
        # one dma_start per PAIR of contraction chunks (host layout
        # [4, 128, 2, CLOC], q weights only -- k weights arrive as fp8)
        t = p_wqk.tile([128, 2, CLOC], CDT, tag="wqk", name=f"wqk{kc0}")
        nc.sync.dma_start(out=t, in_=wqkt[kc0 // 2, :, :, :])
        wqk_sb[kc0] = t[:, 0, :]
        wqk_sb[kc0 + 1] = t[:, 1, :]

    # ---- dense-fill machinery: each fill item emits ONE FULL accumulation
    # chain (8 or 4 back-to-back matmuls + epilogue). Chain granularity
    # keeps the PE instruction stream uniform: scattering single matmuls
    # between attention ops costs a ~130ns restart per insertion. ----
    def qk_chain(g, m):
        # m in 0..7: 0..3 -> q chunks (hp=m), 4..7 -> k chunks (hp=m-4).
        # q chains are bf16 (8 matmuls, 128-contraction each); k chains run
        # fp8e4 DoubleRow (4 matmuls, 256-contraction each -- half the PE
        # time). Only k is quantized: fp8 on both q and k simulates to
        # 2.0e-2 rel err (at the gate), k-only to 1.39e-2.
        def emit(g=g, m=m):
            ps = sh_tile(f"qk{g}_{m}")[:, 0, :]
            if m < 4:
                for kc in range(8):
                    nc.tensor.matmul(ps, wqk_sb[kc][:, m * 128:(m + 1) * 128],
                                     xt_g[g][kc], start=kc == 0, stop=kc == 7,
                                     skip_group_check=True)
                qt = p_q.tile([128, QG], ADT, tag="qsb", name=f"q{g}_{m}")
                nc.vector.tensor_copy(out=qt, in_=ps)
                q_gs[g][m] = qt
            else:
                mk = m - 4
                for kcp in range(4):
                    nc.tensor.matmul(
                        ps, wk8_sb[:, kcp, :, mk * 128:(mk + 1) * 128],
                        x8_g[g][:, 2 * kcp:2 * kcp + 2, :],
                        start=kcp == 0, stop=kcp == 3,
                        perf_mode=DR, skip_group_check=True)
                nc.vector.tensor_copy(
                    out=k_sb[mk][:, g * QG:(g + 1) * QG], in_=ps)
            chains_done.add((g, m))
        return emit

    def v_chain(g, tb):
        def emit(g=g, tb=tb):
            ps = sh_tile(f"v{g}_{tb}")[:, 0, :]
            for kc in range(8):
                nc.tensor.matmul(ps, xt_g[g][kc][:, tb * 128:(tb + 1) * 128],
                                 wv_sb[kc], start=kc == 0, stop=kc == 7,
                                 skip_group_check=True)
            # padded to 128 stationary columns (v | ones | garbage): a full
            # 128-col weight qualifies for FWL, so the AV weight loads hide
            # under the preceding matmul (65-col loads cannot). The ones
            # column was pre-set at startup; cols 65-127 stay uninitialized
            # (their psum partitions 65-127 are never read).
            vp = vp_pre[4 * g + tb]
            nc.vector.tensor_copy(
                out=vp[:, :, 0:64], in_=ps.rearrange("p (h d) -> p h d", d=64))
            vp_sb.append(vp)
        return emit

    ost_pending = {}

    def proj_chain(g, m, copy_on_act=False):
        def emit(g=g, m=m):
            ps = sh_tile(f"pj{g}_{m}")[:, 0, :]
            for c in range(4):
                nc.tensor.matmul(ps, wp_sb[c][:, m * 128:(m + 1) * 128],
                                 yt_gs[g][c], start=c == 0, stop=c == 3,
                                 skip_group_check=True)
            # pair the output stores: one dma_start per two m-chunks
            # (halves the serial descriptor-generation on the sync engine).
            # single_store chains store each chunk on its own so the kernel
            # tail's final store starts as soon as its own chain is done.
            single = g == NG - 1 and m >= 6
            if m % 2 == 0 and not single:
                ost = p_ost.tile([128, 2, QG], CDT, tag="ost",
                                 name=f"ost{g}_{m}")
                ost_pending[g] = ost
            elif single:
                ost = p_ost.tile([128, 2, QG], CDT, tag="ost",
                                 name=f"ost{g}_{m}")
            else:
                ost = ost_pending[g]
            if copy_on_act:
                # keep the in-order DVE queue free for the kernel-tail
                # normalize ops
                nc.scalar.activation(out=ost[:, m % 2, :], in_=ps,
                                     func=mybir.ActivationFunctionType.Copy)
            else:
                nc.vector.tensor_copy(out=ost[:, m % 2, :], in_=ps)
            if single:
                nc.sync.dma_start(out=ot[g, :, m:m + 1, :],
                                  in_=ost[:, m % 2:m % 2 + 1, :])
            elif m % 2 == 1:
                nc.sync.dma_start(out=ot[g, :, m - 1:m + 1, :], in_=ost)
        return emit

    fill = []          # FIFO of pending dense chains
    chains_done = set()

    # ---- PE pre-warm: the HAM clock gate starts at 4/8 (1.2 GHz) and
    # needs ~3.4us of sustained PE busy to release. The first real matmul
    # can't start before ~13us (NEFF init + first DMAs), so burn dummy
    # matmuls that depend only on a memset — the PE is warm before the
    # first chain. ----
    dmy = p_one.tile([1, QG], F32R, tag="dmy")
    nc.vector.memset(dmy.bitcast(F32), 1.0)
    # 24 dummies: the startup DMA stream (4.5MB) only feeds the first real
    # chain at ~18us, and the HAM gate needs sustained PE busy right up to
    # it (15 dummies used to end ~15us and the first chains ran at 1.2GHz)
    for i in range(24):
        psd = sh_tile(f"warm{i}")
        nc.tensor.matmul(psd[0:64, 0, :], ones_sb[0:1, :], dmy,
                         start=True, stop=True, skip_group_check=True)

    # pre-allocate all 16 vp tiles and set their ones column now, while the
    # DVE is idle (startup): the v chains only fill cols 0:64 later, and
    # cols 65:127 stay garbage (their psum partitions are never read).
    vp_pre = []
    for i in range(16):
        vp = p_vp.tile([128, HLOC, 128], ADT, tag="vp", name=f"vp{i}")
        nc.vector.memset(vp[:, :, 64:65], 1.0)
        vp_pre.append(vp)

    # ---- startup: k's fp8 operands first (1MB -> first k chain early),
    # then interleaved wqk/xt0 pair-DMAs ordered by first consumption, wv
    # after (v chains are the first fill); first chains ASAP ----
    wk8_sb = p_wk8.tile([128, 4, 2, CLOC], FP8, tag="wk8")
    nc.sync.dma_start(out=wk8_sb, in_=wk8d[:, :, :, :])
    dma_x8(0)
    for kc0 in range(0, 8, 2):
        dma_wqk(kc0)
        dma_x(0, [kc0, kc0 + 1], p_xt0)
    # wv after the full wqk/xt0 stream: the first q/k chain contracts over
    # ALL kc chunks, while the v chains (fill) only start at the first AVs
    wv_t = p_wv.tile([128, 8, CLOC], CDT, tag="wv")
    nc.sync.dma_start(out=wv_t, in_=wvt[:, :, :])
    for kc in range(8):
        wv_sb[kc] = wv_t[:, kc, :]
    nc.sync.dma_start(out=tri_sb, in_=tri[:, :])

    with nc.named_scope("qkv0"):
        qk_chain(0, 4)()
        qk_chain(0, 0)()

    # ---- main loop over query groups ----
    pending_norm = []   # deferred normalize phase-B closures (max 1 deep)
    for g in range(NG):
        if 0 < g < NG - 1:
            dma_x8(g + 1)
            dma_x(g + 1, list(range(8)), p_xt)

        # fill supply for this group's attention (deadline-ordered):
        #   [g=0 only: v0 + the rest of qkv0], qkv(g+1) q/k + v(g+1).
        # proj has no deadline before the kernel end, so ALL proj chains
        # are deferred to attn3 (the group with by far the most ACT-paced
        # attention to fill) — minus a few reserved for the kernel tail to
        # keep the PE busy (and HAM warm) through the last normalize.
        if g == 0:
            for tb in range(4):
                fill.append(v_chain(0, tb))
            for m in (1, 5, 2, 6, 3, 7):
                fill.append(qk_chain(0, m))
        if g + 1 < NG:
            for m in (0, 4, 1, 5, 2, 6, 3, 7):
                fill.append(qk_chain(g + 1, m))
            if g + 1 < NG - 1:
                for tb in range(4):
                    fill.append(v_chain(g + 1, tb))
        if g == NG - 1:
            for tb in range(4):
                fill.append(v_chain(g, tb))
            for gp in (0, 1, 2):
                for m in range(8):
                    fill.append(proj_chain(gp, m))
            # reserves keep the PE busy through the final normalize; their
            # psum->sbuf copies go on the ACT engine so the DVE queue
            # stays clear for the normalize chain
            tail_reserve = [proj_chain(2, m, copy_on_act=True)
                            for m in (3, 4, 5, 6, 7)]
            del fill[-5:]

        K_g = 4 * (g + 1)
        # fill rate in CHAINS per attention block (retuned after the fp8 k
        # chains halved: shorter chains -> more pops per pair to keep the
        # exp-paced attention gaps filled)
        rate = [float(v) for v in os.environ.get(
            "ATTN_RATES", "1.45,0.4,0.18,0.42").split(",")][g]
        budget = 0.0
        with nc.named_scope(f"attn{g}"):
            yt_g = [p_yt.tile([128, QG], CDT, tag="yt", name=f"yt{g}_{c}")
                    for c in range(4)]
            yt_gs[g] = yt_g
            for hp in range(4):
                if g == 0 and hp == 1:
                    # group-0 prefetch DMAs deferred past hp0 so their
                    # descriptor generation stays off the startup critical
                    # path on the sync engine.
                    dma_x8(1)
                    dma_x(1, list(range(8)), p_xt)
                    wp_t = p_wp.tile([128, 4, C], CDT, tag="wp")
                    nc.sync.dma_start(out=wp_t, in_=wpt[:, :, :])
                    for kc in range(4):
                        wp_sb.append(wp_t[:, kc, :])
                # the in-order PE queue deadlocks on any backward
                # dependency: this hp's q/k chains must be fully EMITTED
                # before its first S matmul (only group 0 has the
                # intra-group deadline).
                while ((g, hp) not in chains_done
                       or (g, 4 + hp) not in chains_done):
                    fill.pop(0)()
                psy = [ps_y.tile([128, QG], F32, tag="psy",
                                 name=f"psy{g}_{hp}_{r}") for r in range(2)]
                # software pipeline: emit S(kb), fill, AV(kb-1)
                es_q = []   # (kb, es tile)

                def emit_S(kb, hp=hp, g=g):
                    j = kb - 4 * g
                    c0 = max(0, 128 * j)
                    vis = slice(c0, QG)
                    ps = sh_tile(f"pss{g}_{hp}_{kb}")
                    for r in (0, 1):
                        row = slice(64 * r, 64 * r + 64)
                        nc.tensor.matmul(
                            ps[:, r, vis],
                            k_sb[hp][row, kb * 128:(kb + 1) * 128],
                            q_gs[g][hp][row, vis], start=True, stop=True,
                            skip_group_check=True)
                    es = p_es.tile([128, 2, QG], ADT, tag="es")
                    nc.scalar.activation(out=es[:, :, vis], in_=ps[:, :, vis],
                                         func=EXP, scale=SCALE)
                    if j >= 0:
                        # one mask multiply covering BOTH heads (tri
                        # broadcast over the r dim): these ops are
                        # overhead-dominated, so halving the count saves
                        # ~200ns per diagonal block on the DVE queue
                        nc.vector.tensor_mul(
                            es[:, :, c0:c0 + 128], es[:, :, c0:c0 + 128],
                            tri_sb.rearrange("p (o t) -> p o t",
                                             o=1).to_broadcast([KB, 2, KB]))
                    es_q.append((kb, es))

                def emit_AV(hp=hp, g=g, K_g=K_g):
                    kb, es = es_q.pop(0)
                    j = kb - 4 * g
                    c0 = max(0, 128 * j)
                    vis = slice(c0, QG)
                    # the v chain producing vp_sb[kb] must already be
                    # emitted (in-order PE queue): force-drain fill if not
                    while len(vp_sb) <= kb:
                        fill.pop(0)()
                    for r in (0, 1):
                        h = 2 * hp + r
                        nc.tensor.matmul(psy[r][:, vis],
                                         vp_sb[kb][:, h, :],
                                         es[:, r, vis], start=kb == 0,
                                         stop=kb == K_g - 1,
                                         skip_group_check=True)

                # stride-2 software pipeline: [S(b), S(b+1)] [fill]
                # [AV(b-2), AV(b-1)] — S leads AV by two blocks so the AVs
                # never wait on exp, and the pair batching halves the
                # per-insertion PE restart cost.
                for base in range(0, K_g, 2):
                    budget += 2 * rate
                    emit_S(base)
                    emit_S(base + 1)
                    if base == (4 if K_g > 4 else 2) and pending_norm:
                        # flush the PREVIOUS hp's deferred normalize here:
                        # by now its sums input is ~4us old, so the Ln never
                        # suspends the in-order ACT queue waiting on DVE
                        # (which used to block ready attention exps ~2us and
                        # stall the PE on psum-slot reuse).
                        pending_norm.pop(0)()
                    while budget >= 1.0 and fill:
                        fill.pop(0)()
                        budget -= 1.0
                    while len(es_q) > 2:
                        emit_AV()
                while es_q:
                    emit_AV()

                tail = g == NG - 1 and hp == 3
                # normalize off the PE critical path: psum -> sbuf, the two
                # heads' denominators batched into single Ln/Exp ACT ops,
                # DMA broadcast, scale on gpsimd (so the broadcast's DMA
                # roundtrip latency never head-of-line-blocks the DVE
                # queue, which gates chain psum reuse and the tri masks).
                # denominators FIRST, straight from psum row 64 (2 tiny
                # copies): sums is then ready ~3us before the deferred Ln
                # reaches the ACT queue head, so the in-order ACT queue
                # never suspends waiting on it (it used to sit behind the
                # big ysb/fill casts on the DVE queue)
                sums = p_rec.tile([33, QG], F32R, tag="sums",
                                  name=f"sm{g}_{hp}")
                for r in (0, 1):
                    nc.vector.tensor_copy(out=sums[32 * r:32 * r + 1, :],
                                          in_=psy[r][64:65, :])
                ysbs = []
                for r in (0, 1):
                    ysb = p_ysb.tile([65, QG], F32R, tag="ysb",
                                     name=f"ysb{g}_{hp}_{r}")
                    nc.vector.tensor_copy(out=ysb, in_=psy[r][0:65, :])
                    ysbs.append(ysb)
                # reciprocal via Ln + Exp(-x) on ACT: the native DVE
                # InstReciprocal takes ~3.3us per [33,512] tile and
                # head-of-line blocks the DVE queue (masks, chain copies) --
                # measured +25us end-to-end vs this two-op ACT path.
                if tail:
                    lns = p_rec.tile([33, QG], F32, tag="lns",
                                     name=f"ln{g}_{hp}")
                    nc.scalar.activation(out=lns, in_=sums.bitcast(F32),
                                         func=mybir.ActivationFunctionType.Ln)
                    rec = p_rec.tile([33, QG], F32R, tag="rec",
                                     name=f"rec{g}_{hp}")
                    nc.scalar.activation(out=rec, in_=lns, func=EXP,
                                         scale=-1.0)
                    # keep the PE busy (and HAM warm) through the final
                    # normalize chain; emitted after the normalize's DVE/ACT
                    # ops so those queues stay clear of reserve work
                    for ch in tail_reserve:
                        ch()
                    # kernel tail: PE broadcast straight from rec, and the
                    # yt multiply reads the PSUM result directly — the DMA
                    # roundtrip latency would gate proj3
                    psbs = []
                    for r in (0, 1):
                        psb = sh_tile(f"psbx{r}")
                        nc.tensor.matmul(
                            psb[0:64, 0, :],
                            ones_sb[32 * r:32 * r + 1, :],
                            rec[32 * r:32 * r + 1, :],
                            start=True, stop=True, skip_group_check=True)
                        psbs.append(psb)
                    # both tail muls on DVE: gpsimd cannot read PSUM (psb)
                    for r in (0, 1):
                        nc.vector.tensor_mul(yt_g[hp][64 * r:64 * r + 64, :],
                                             ysbs[r][0:64, :].bitcast(F32),
                                             psbs[r][0:64, 0, :])
                else:
                    def norm_b(g=g, hp=hp, sums=sums, ysbs=ysbs,
                               yt_hp=yt_g[hp]):
                        lns = p_rec.tile([33, QG], F32, tag="lns",
                                         name=f"ln{g}_{hp}")
                        nc.scalar.activation(
                            out=lns, in_=sums.bitcast(F32),
                            func=mybir.ActivationFunctionType.Ln)
                        rec = p_rec.tile([33, QG], F32R, tag="rec",
                                         name=f"rec{g}_{hp}")
                        nc.scalar.activation(out=rec, in_=lns, func=EXP,
                                             scale=-1.0)
                        bc = p_bc.tile([64, 2, QG], F32, tag="bc",
                                       name=f"bc{g}_{hp}")
                        recd = p_drb.tile([33, QG], F32, tag="recd",
                                          name=f"recd{g}_{hp}")
                        nc.sync.dma_start(out=recd, in_=rec.bitcast(F32))
                        # single broadcast DMA for both heads:
                        # row 32r -> 64 rows
                        nc.sync.dma_start(
                            out=bc,
                            in_=recd[0:33:32, :].rearrange(
                                "(o j) t -> o j t",
                                o=1).to_broadcast([64, 2, QG]))
                        mul_eng = nc.vector if MUL == "dve" else nc.gpsimd
                        for r in (0, 1):
                            mul_eng.tensor_mul(yt_hp[64 * r:64 * r + 64, :],
                                               ysbs[r][0:64, :].bitcast(F32),
                                               bc[:, r, :])
                    pending_norm.append(norm_b)
        while fill:
            fill.pop(0)()

    with nc.named_scope("proj3"):
        for m in range(8):
            # alternate the psum->sbuf copies across ACT and DVE so the
            # final stores drain without queue backlog
            proj_chain(NG - 1, m, copy_on_act=m % 2 == 0)()


def _build_nc():
    from contextlib import ExitStack
    nc = bass.Bass(trn_type="TRN2")
    # all tensors host-pre-tiled so every DMA has >=2KB contiguous
    # per-partition runs
    xt = nc.dram_tensor("xt", [NG, 128, 8, QG], CDT, kind="ExternalInput")
    x8d = nc.dram_tensor("x8", [NG, 128, 8, QG], FP8, kind="ExternalInput")
    wqkt = nc.dram_tensor("wqkt", [4, 128, 2, CLOC], CDT,
                          kind="ExternalInput")
    wk8d = nc.dram_tensor("wk8", [128, 4, 2, CLOC], FP8,
                          kind="ExternalInput")
    wvt = nc.dram_tensor("wvt", [128, 8, CLOC], CDT, kind="ExternalInput")
    wpt = nc.dram_tensor("wpt", [128, 4, C], CDT, kind="ExternalInput")
    tri = nc.dram_tensor("tri", [KB, KB], CDT, kind="ExternalInput")
    ot = nc.dram_tensor("ot", [NG, 128, 8, QG], CDT, kind="ExternalOutput")
    with tile.TileContext(nc) as tc:
        with ExitStack() as ctx:
            _build_body(nc, tc, ctx, xt, x8d, wqkt, wk8d, wvt, wpt, tri, ot)
    return nc


LAST_RESULTS = None
_NC_CACHE = None


def kernel(x, W_qkv, W_proj):
    global LAST_RESULTS, _NC_CACHE
    x = np.asarray(x, dtype=np.float32)
    W_qkv = np.asarray(W_qkv, dtype=np.float32)
    W_proj = np.asarray(W_proj, dtype=np.float32)

    if _NC_CACHE is None:
        _NC_CACHE = _build_nc()
    nc = _NC_CACHE
    import ml_dtypes
    tri = np.triu(np.ones((KB, KB), np.float32))
    in_maps = []
    for core in range(8):
        b, hg = core // 2, core % 2
        rq = slice(CLOC * hg, CLOC * hg + CLOC)
        Wq = W_qkv[0:C][rq]
        Wk = W_qkv[C:2 * C][rq]
        Wv = W_qkv[2 * C:3 * C][rq]
        # x[b].T is [C, T]: tile to [NG, 128, 8, QG] with
        # xt[g, p, kc, t] = x.T[kc*128+p, g*QG+t]
        xtb = x[b].T.reshape(8, 128, NG, QG).transpose(2, 1, 0, 3)
        wq4 = Wq.T.reshape(4, 2, 128, CLOC).transpose(0, 2, 1, 3)
        # k weights go fp8e4 (DoubleRow): pre-scale by KSC=64 so the
        # uniform(-1/32,1/32) values sit in e4m3's normal range; the 1/64
        # is folded into the exp scale on S
        wk8 = ((Wk.T * KSC).reshape(4, 2, 128, CLOC)
               .transpose(2, 0, 1, 3))       # [128, 4, 2, CLOC]
        wv8 = Wv.T.reshape(8, 128, CLOC).transpose(1, 0, 2)
        wp4 = W_proj[:, rq].T.reshape(4, 128, C).transpose(1, 0, 2)
        _c = lambda a: np.ascontiguousarray(a).astype(ml_dtypes.bfloat16)
        _c8 = lambda a: np.ascontiguousarray(a).astype(ml_dtypes.float8_e4m3)
        in_maps.append({
            "xt": _c(xtb),
            "x8": _c8(xtb),
            "wqkt": _c(wq4),
            "wk8": _c8(wk8),
            "wvt": _c(wv8),
            "wpt": _c(wp4),
            "tri": _c(tri),
        })

    trace = os.environ.get("ATTN_BASS_TRACE") == "1"
    last_exc = None
    out = None
    for attempt in range(4):
        try:
            res = run_bass_kernel_spmd(nc, in_maps, core_ids=list(range(8)),
                                       trace=trace)
        except Exception as e:  # transient NRT device errors happen
            last_exc = e
            import time as _time
            _time.sleep(2.0)
            continue
        LAST_RESULTS = res
        out = np.empty((B, T, C), np.float32)
        for b in range(B):
            acc = (res.results[2 * b]["ot"].astype(np.float32)
                   + res.results[2 * b + 1]["ot"].astype(np.float32))
            # [NG, 128, 8, QG] -> [C, T] -> [T, C]
            out[b] = acc.transpose(2, 1, 0, 3).reshape(C, T).T
        # rare transient device flakes can corrupt a run: retry on any
        # non-finite output
        if np.isfinite(out).all():
            return out
    if out is None:
        raise last_exc
    return out



# revision 51
# speedup vs baseline: 1.0099x; 1.0099x over previous
"""Trainium2 Bass kernel for NanoAttention (B=4, T=2048, C=1024, H=16, causal).

Sharding: 8 cores = 4 batches x 2 head-groups (8 heads each).
Per core (b, hg):
  - column-parallel qkv:  q,k produced in [channel, token] (transposed) layout,
    v produced in [token, channel] (natural) layout with an appended ones
    column per head (fuses the softmax denominator into the AV matmul).
  - causal attention per head with S^T in [key, query] layout; exp on ACT with
    scale=1/sqrt(D); block-causal skipping + triangular masking on the
    diagonal blocks.
  - row-parallel proj producing a partial output [C, T] in bf16; the host adds
    the two head-group partials per batch and transposes back.

Session-2 optimizations (271us -> ~256us min / ~257us median):
  - k projection in fp8e4 DoubleRow: W_k host-prescaled by 64 (fp8e4
    normal range; 1/64 folded into the exp scale) and x in fp8; each k
    chain is 4 matmuls of 256-contraction instead of 8 of 128 (half the
    PE time; DoubleRow measured at the SAME ns/instr as bf16 = 2x MACs,
    NOT the cost model's 0.5 cyc/row). fp8 on BOTH q and k sims to
    2.0e-2 rel err (at the gate); k-only gives 1.2e-2 measured.
  - one shared 3-slot psum pool (6 banks) for S blocks + fill chains
    (deeper rotation absorbs ACT/DVE latency spikes).
  - per-hp normalize split: denominators copied psum->sbuf FIRST (2 tiny
    copies, so sums is ready early), then Ln/Exp/broadcast/muls deferred
    into the NEXT hp's pair loop -- the in-order ACT queue used to
    suspend ~2-3us at the Ln waiting on DVE-queued sums, blocking ready
    attention exps and stalling the PE on psum-slot reuse.
  - diag tri-masks cover both heads in one DVE op (tri broadcast over r).
  - vp tiles pre-allocated at startup (ones column memset in idle time,
    zero-pad dropped: garbage cols 65:127 feed psum partitions nobody
    reads).
  - the last two output stores are unpaired so the final DMA launches as
    soon as its own chain lands.
  Failed experiments (recorded so they are not retried): DVE
  InstReciprocal = 3.3us/[33,512] (queue-blocking; Ln+Exp on ACT is far
  better, and ACT-table Reciprocal is in a different table set than Exp
  so it would pay 1.3us table switches); dma_start on the ACT HWDGE
  queue corrupts input data in this container; a single static psum tile
  with python-side slot rotation (for paired exps over adjacent slots)
  serializes the kernel to ~490us (subview dependency tracking is
  tile-granular); exp pairing via pool tiles is psum-capacity-blocked;
  hi/lo fp8 splits lose (3 terms x 2x-MACs > bf16); fill rebalancing
  (v(3) into attn2 etc.) only relocates the starvation.

Optimizations vs the 319us v1 baseline (session 1, ~271us, ~15%):
  - software-pipelined emission with a stride-2 rotation: the PE queue is
    in-order, so [S(b), S(b+1)] [dense fill] [AV(b-2), AV(b-1)] keeps the
    AVs off the exp critical path and the pair batching halves the ~130ns
    per-insertion PE restart cost.
  - dense work (qkv of the next group, v chains, ALL proj chains) is
    drip-fed between attention blocks in whole-chain granularity; proj is
    deferred to attn3, which has by far the most ACT(exp)-paced attention
    to fill; 5 chains are reserved for the kernel tail so the PE (and the
    HAM clock gate) stays busy through the final normalize.
  - every dma_start costs ~630ns of serial descriptor generation on the
    sync engine: inputs/outputs are host-pre-tiled ([128, n, free] with
    >=2KB contiguous per-partition runs) and loaded/stored with few
    multi-chunk transfers; output stores are paired.
  - PE pre-warm: ~15 dummy matmuls gated only on a memset lift the HAM
    clock gate from 4/8 (1.2GHz) to 8/8 (2.4GHz) before the first real
    matmul's DMA dependencies can possibly arrive (~13us).
  - vp stationaries padded to 128 columns (v | ones | zeros): full-width
    weights qualify for FWL so AV weight loads hide under the previous
    matmul (65-wide loads cannot and cost ~+120ns per AV pair).
  - the softmax reciprocal broadcast goes SBUF->DRAM->broadcast-DMA, and
    the yt normalize multiplies run on gpsimd so the roundtrip latency
    never head-of-line-blocks the in-order DVE queue (which gates dense
    chain psum reuse and the causal tri masks); the kernel-tail broadcast
    uses a PE ones-matmul directly from rec with the multiply reading
    PSUM, skipping two copies.
  - output DMA in bf16 (host accumulates the two head-group partials in
    f32; rel err unaffected at 4.3e-3 vs the 2e-2 gate).
  (gpsimd partition_broadcast and the custom-DVE reciprocal_approx_fast
  do not compile in this container's walrus: "ISA wrong length".)
"""
import os
import sys

sys.path.insert(0, '/opt/trn_rl_repo')

import numpy as np
import orjson

import concourse.bass as bass
import concourse.mybir as mybir
import concourse.tile as tile
from concourse.bass_utils import run_bass_kernel_spmd

# ---------------------------------------------------------------------------
# Workaround for this container's walrus build: it enforces the HW limit of
# one sync-wait per instruction (two for EventSemaphore), but Tile's sem
# assignment can emit more (kernel-tail Drain waits on every DMA queue used;
# HWDGE stores can pick up two queue waits). Split the overflow onto
# preceding pure-wait EventSemaphore instructions on the same engine at
# JSON-serialization time so every compile path is covered.
# ---------------------------------------------------------------------------


def _split_multi_waits(data):
    n_split = 0
    for func in data.get("functions", []):
        for blk in func.get("blocks", []):
            insts = blk.get("instructions")
            if not insts:
                continue
            out = []
            for inst in insts:
                si = inst.get("sync_info")
                waits = (si or {}).get("on_wait") or []
                cap = 2 if inst.get("opcode") == "EventSemaphore" else 1
                if len(waits) > cap and "engine" in inst:
                    extra = waits[:-cap]
                    si["on_wait"] = waits[-cap:]
                    for i in range(0, len(extra), 2):
                        n_split += 1
                        out.append({
                            "debug": inst.get("debug"),
                            "engine": inst["engine"],
                            "ins": [],
                            "outs": [],
                            "name": f"{inst['name']}_wsplit{n_split}",
                            "opcode": "EventSemaphore",
                            "sync_info": {"on_wait": extra[i:i + 2],
                                          "on_update": []},
                        })
                out.append(inst)
            blk["instructions"] = out
    return data


_orig_to_json_bytes = bass.Bass.to_json_bytes


def _patched_to_json_bytes(self):
    return orjson.dumps(_split_multi_waits(orjson.loads(_orig_to_json_bytes(self))))


bass.Bass.to_json_bytes = _patched_to_json_bytes

# ---------------------------------------------------------------------------

B, T, C = 4, 2048, 1024
N_HEAD, D = 16, 64
HLOC = 8          # heads per core
CLOC = HLOC * D   # 512 local qkv channels per core
QG = 512          # query-group width
NG = T // QG      # 4 query groups
KB = 128          # key-block width
F32R = mybir.dt.float32r
F32 = mybir.dt.float32
BF16 = mybir.dt.bfloat16
CDT = BF16
ADT = CDT
FP8 = mybir.dt.float8e4
DR = mybir.MatmulPerfMode.DoubleRow
KSC = 64.0        # host pre-scale on W_k so fp8e4 stays in normal range
EXP = mybir.ActivationFunctionType.Exp
SCALE = 1.0 / np.sqrt(D) / KSC   # folds the k fp8 pre-scale out of S
# reciprocal broadcast path: "dma" (DRAM roundtrip broadcast) or "gpsimd"
# (partition_broadcast — does NOT compile in this container's walrus:
# "ISA wrong length", same for the custom-DVE reciprocal_approx_fast).
BCAST = os.environ.get("ATTN_BCAST", "dma")
# engine for the yt normalize multiplies: "gpsimd" keeps the DMA-broadcast
# wait out of the DVE queue; "dve" is the fallback.
MUL = os.environ.get("ATTN_MUL", "gpsimd")


def _build_body(nc, tc, ctx, xt, x8d, wqkt, wk8d, wvt, wpt, tri, ot):
    p_wqk = ctx.enter_context(tc.tile_pool(name="wqk", bufs=4))
    p_wk8 = ctx.enter_context(tc.tile_pool(name="wk8", bufs=1))
    p_x8 = ctx.enter_context(tc.tile_pool(name="x8", bufs=2))
    p_wv = ctx.enter_context(tc.tile_pool(name="wv", bufs=1))
    p_wp = ctx.enter_context(tc.tile_pool(name="wp", bufs=1))
    p_xt0 = ctx.enter_context(tc.tile_pool(name="xt0", bufs=4))
    p_xt = ctx.enter_context(tc.tile_pool(name="xt", bufs=2))
    p_k = ctx.enter_context(tc.tile_pool(name="ksb", bufs=4))
    p_q = ctx.enter_context(tc.tile_pool(name="qsb", bufs=8))
    p_vp = ctx.enter_context(tc.tile_pool(name="vp", bufs=16))
    p_es = ctx.enter_context(tc.tile_pool(name="es", bufs=7))
    # all four groups' yt stay alive: proj is deferred to attn3/tail
    p_yt = ctx.enter_context(tc.tile_pool(name="yt", bufs=16))
    p_ost = ctx.enter_context(tc.tile_pool(name="ost", bufs=4))
    p_one = ctx.enter_context(tc.tile_pool(name="one", bufs=1))
    p_rec = ctx.enter_context(tc.tile_pool(name="rec", bufs=3))
    p_ysb = ctx.enter_context(tc.tile_pool(name="ysb", bufs=4))
    p_bc = ctx.enter_context(tc.tile_pool(name="bc", bufs=2))
    if BCAST == "dma":
        p_drb = ctx.enter_context(tc.tile_pool(name="drb", bufs=2, space="DRAM"))
    # ONE shared 3-slot psum pool (6 banks) for S blocks AND dense-fill
    # chains: the extra rotation depth gives the S exp (and fill copies)
    # ~2-3us of slack before their slot is reused, absorbing ACT/DVE queue
    # latency spikes that used to stall the in-order PE queue ~1.4us at a
    # time with the old 2-buf ps_s.
    # NOTE: a single static [128,3,2,QG] psum tile with python-side slot
    # rotation (to enable paired exps over adjacent slots) serializes the
    # whole kernel to ~490us: the tile framework's dependency tracking on
    # subviews of one tile is far more conservative than across pool tiles.
    # Keep pool-based rotation.
    ps_sh = ctx.enter_context(tc.tile_pool(name="pssh", bufs=3, space="PSUM"))
    ps_y = ctx.enter_context(tc.tile_pool(name="psy", bufs=2, space="PSUM"))

    def sh_tile(name):
        return ps_sh.tile([128, 2, QG], F32, tag="pss", name=name)

    # ---- static state ----
    # Each dma_start costs ~630ns of serial descriptor-generation on the
    # sync engine, so inputs are loaded with FEW multi-chunk transfers:
    # contraction-chunk views are AP slices of [128, n, free] tiles.
    wqk_sb = [None] * 8   # views: wqk_sb[kc] = [128, CLOC] (q-only, bf16)
    wv_sb = [None] * 8
    wp_sb = []
    xt_g = [[None] * 8 for _ in range(NG)]
    x8_g = [None] * NG    # [128, 8, QG] fp8 tiles (k-chain moving operand)
    wk8_sb = None         # [128, 4, 2, CLOC] fp8 (k DoubleRow stationaries)
    tri_sb = p_one.tile([KB, KB], CDT, tag="tri")
    ones_sb = p_one.tile([33, 64], F32R, tag="ones")
    nc.vector.memset(ones_sb.bitcast(F32), 1.0)
    k_sb = [p_k.tile([128, T], ADT, tag="ksb", name=f"ksb{c}") for c in range(4)]
    q_gs = [[None] * 4 for _ in range(NG)]
    vp_sb = []        # grows to 16 tiles, 4 per group
    yt_gs = [None] * NG

    def dma_x(g, kcs, pool, eng=None):
        # one dma_start covering contraction chunks kcs (list); the host
        # pre-tiles x as [NG, 128, 8, QG] so each partition's run is
        # contiguous (n*1KB descriptors instead of n*128)
        n = len(kcs)
        t = pool.tile([128, n, QG], CDT, tag="xt", name=f"xt{g}_{kcs[0]}")
        (eng or nc.sync).dma_start(out=t, in_=xt[g, :, kcs[0]:kcs[0] + n, :])
        for i, kc in enumerate(kcs):
            xt_g[g][kc] = t[:, i, :]

    def dma_x8(g):
        t = p_x8.tile([128, 8, QG], FP8, tag="x8", name=f"x8_{g}")
        nc.sync.dma_start(out=t, in_=x8d[g, :, :, :])
        x8_g[g] = t

    def dma_wqk(kc0):
        # one dma_start per PAIR of contraction chunks (host layout
        # [4, 128, 2, CLOC], q weights only -- k weights arrive as fp8)
        t = p_wqk.tile([128, 2, CLOC], CDT, tag="wqk", name=f"wqk{kc0}")
        nc.sync.dma_start(out=t, in_=wqkt[kc0 // 2, :, :, :])
        wqk_sb[kc0] = t[:, 0, :]
        wqk_sb[kc0 + 1] = t[:, 1, :]

    # ---- dense-fill machinery: each fill item emits ONE FULL accumulation
    # chain (8 or 4 back-to-back matmuls + epilogue). Chain granularity
    # keeps the PE instruction stream uniform: scattering single matmuls
    # between attention ops costs a ~130ns restart per insertion. ----
    def qk_chain(g, m):
        # m in 0..7: 0..3 -> q chunks (hp=m), 4..7 -> k chunks (hp=m-4).
        # q chains are bf16 (8 matmuls, 128-contraction each); k chains run
        # fp8e4 DoubleRow (4 matmuls, 256-contraction each -- half the PE
        # time). Only k is quantized: fp8 on both q and k simulates to
        # 2.0e-2 rel err (at the gate), k-only to 1.39e-2.
        def emit(g=g, m=m):
            ps = sh_tile(f"qk{g}_{m}")[:, 0, :]
            if m < 4:
                for kc in range(8):
                    nc.tensor.matmul(ps, wqk_sb[kc][:, m * 128:(m + 1) * 128],
                                     xt_g[g][kc], start=kc == 0, stop=kc == 7,
                                     skip_group_check=True)
                qt = p_q.tile([128, QG], ADT, tag="qsb", name=f"q{g}_{m}")
                nc.vector.tensor_copy(out=qt, in_=ps)
                q_gs[g][m] = qt
            else:
                mk = m - 4
                for kcp in range(4):
                    nc.tensor.matmul(
                        ps, wk8_sb[:, kcp, :, mk * 128:(mk + 1) * 128],
                        x8_g[g][:, 2 * kcp:2 * kcp + 2, :],
                        start=kcp == 0, stop=kcp == 3,
                        perf_mode=DR, skip_group_check=True)
                nc.vector.tensor_copy(
                    out=k_sb[mk][:, g * QG:(g + 1) * QG], in_=ps)
            chains_done.add((g, m))
        return emit

    def v_chain(g, tb):
        def emit(g=g, tb=tb):
            ps = sh_tile(f"v{g}_{tb}")[:, 0, :]
            for kc in range(8):
                nc.tensor.matmul(ps, xt_g[g][kc][:, tb * 128:(tb + 1) * 128],
                                 wv_sb[kc], start=kc == 0, stop=kc == 7,
                                 skip_group_check=True)
            # padded to 128 stationary columns (v | ones | garbage): a full
            # 128-col weight qualifies for FWL, so the AV weight loads hide
            # under the preceding matmul (65-col loads cannot). The ones
            # column was pre-set at startup; cols 65-127 stay uninitialized
            # (their psum partitions 65-127 are never read).
            vp = vp_pre[4 * g + tb]
            nc.vector.tensor_copy(
                out=vp[:, :, 0:64], in_=ps.rearrange("p (h d) -> p h d", d=64))
            vp_sb.append(vp)
        return emit

    ost_pending = {}

    def proj_chain(g, m, copy_on_act=False):
        def emit(g=g, m=m):
            ps = sh_tile(f"pj{g}_{m}")[:, 0, :]
            for c in range(4):
                nc.tensor.matmul(ps, wp_sb[c][:, m * 128:(m + 1) * 128],
                                 yt_gs[g][c], start=c == 0, stop=c == 3,
                                 skip_group_check=True)
            # pair the output stores: one dma_start per two m-chunks
            # (halves the serial descriptor-generation on the sync engine).
            # single_store chains store each chunk on its own so the kernel
            # tail's final store starts as soon as its own chain is done.
            single = g == NG - 1 and m >= 6
            if m % 2 == 0 and not single:
                ost = p_ost.tile([128, 2, QG], CDT, tag="ost",
                                 name=f"ost{g}_{m}")
                ost_pending[g] = ost
            elif single:
                ost = p_ost.tile([128, 2, QG], CDT, tag="ost",
                                 name=f"ost{g}_{m}")
            else:
                ost = ost_pending[g]
            if copy_on_act:
                # keep the in-order DVE queue free for the kernel-tail
                # normalize ops
                nc.scalar.activation(out=ost[:, m % 2, :], in_=ps,
                                     func=mybir.ActivationFunctionType.Copy)
            else:
                nc.vector.tensor_copy(out=ost[:, m % 2, :], in_=ps)
            if single:
                nc.sync.dma_start(out=ot[g, :, m:m + 1, :],
                                  in_=ost[:, m % 2:m % 2 + 1, :])
            elif m % 2 == 1:
                nc.sync.dma_start(out=ot[g, :, m - 1:m + 1, :], in_=ost)
        return emit

    fill = []          # FIFO of pending dense chains
    chains_done = set()

    # ---- PE pre-warm: the HAM clock gate starts at 4/8 (1.2 GHz) and
    # needs ~3.4us of sustained PE busy to release. The first real matmul
    # can't start before ~13us (NEFF init + first DMAs), so burn dummy
    # matmuls that depend only on a memset — the PE is warm before the
    # first chain. ----
    dmy = p_one.tile([1, QG], F32R, tag="dmy")
    nc.vector.memset(dmy.bitcast(F32), 1.0)
    # 24 dummies: the startup DMA stream (4.5MB) only feeds the first real
    # chain at ~18us, and the HAM gate needs sustained PE busy right up to
    # it (15 dummies used to end ~15us and the first chains ran at 1.2GHz)
    for i in range(24):
        psd = sh_tile(f"warm{i}")
        nc.tensor.matmul(psd[0:64, 0, :], ones_sb[0:1, :], dmy,
                         start=True, stop=True, skip_group_check=True)

    # pre-allocate all 16 vp tiles and set their ones column now, while the
    # DVE is idle (startup): the v chains only fill cols 0:64 later, and
    # cols 65:127 stay garbage (their psum partitions are never read).
    vp_pre = []
    for i in range(16):
        vp = p_vp.tile([128, HLOC, 128], ADT, tag="vp", name=f"vp{i}")
        nc.vector.memset(vp[:, :, 64:65], 1.0)
        vp_pre.append(vp)

    # ---- startup: k's fp8 operands first (1MB -> first k chain early),
    # then interleaved wqk/xt0 pair-DMAs ordered by first consumption, wv
    # after (v chains are the first fill); first chains ASAP ----
    wk8_sb = p_wk8.tile([128, 4, 2, CLOC], FP8, tag="wk8")
    nc.sync.dma_start(out=wk8_sb, in_=wk8d[:, :, :, :])
    dma_x8(0)
    for kc0 in range(0, 8, 2):
        dma_wqk(kc0)
        dma_x(0, [kc0, kc0 + 1], p_xt0)
    # wv after the full wqk/xt0 stream: the first q/k chain contracts over
    # ALL kc chunks, while the v chains (fill) only start at the first AVs
    wv_t = p_wv.tile([128, 8, CLOC], CDT, tag="wv")
    nc.sync.dma_start(out=wv_t, in_=wvt[:, :, :])
    for kc in range(8):
        wv_sb[kc] = wv_t[:, kc, :]
    nc.sync.dma_start(out=tri_sb, in_=tri[:, :])

    with nc.named_scope("qkv0"):
        qk_chain(0, 4)()
        qk_chain(0, 0)()

    # ---- main loop over query groups ----
    pending_norm = []   # deferred normalize phase-B closures (max 1 deep)
    for g in range(NG):
        if 0 < g < NG - 1:
            dma_x8(g + 1)
            dma_x(g + 1, list(range(8)), p_xt)

        # fill supply for this group's attention (deadline-ordered):
        #   [g=0 only: v0 + the rest of qkv0], qkv(g+1) q/k + v(g+1).
        # proj has no deadline before the kernel end, so ALL proj chains
        # are deferred to attn3 (the group with by far the most ACT-paced
        # attention to fill) — minus a few reserved for the kernel tail to
        # keep the PE busy (and HAM warm) through the last normalize.
        if g == 0:
            for tb in range(4):
                fill.append(v_chain(0, tb))
            for m in (1, 5, 2, 6, 3, 7):
                fill.append(qk_chain(0, m))
        if g + 1 < NG:
            for m in (0, 4, 1, 5, 2, 6, 3, 7):
                fill.append(qk_chain(g + 1, m))
            if g + 1 < NG - 1:
                for tb in range(4):
                    fill.append(v_chain(g + 1, tb))
        if g == NG - 1:
            for tb in range(4):
                fill.append(v_chain(g, tb))
            for gp in (0, 1, 2):
                for m in range(8):
                    fill.append(proj_chain(gp, m))
            # reserves keep the PE busy through the final normalize; their
            # psum->sbuf copies go on the ACT engine so the DVE queue
            # stays clear for the normalize chain
            tail_reserve = [proj_chain(2, m, copy_on_act=True)
                            for m in (4, 5, 6, 7)]
            del fill[-4:]

        K_g = 4 * (g + 1)
        # fill rate in CHAINS per attention block (retuned after the fp8 k
        # chains halved: shorter chains -> more pops per pair to keep the
        # exp-paced attention gaps filled)
        rate = [float(v) for v in os.environ.get(
            "ATTN_RATES", "1.45,0.4,0.18,0.42").split(",")][g]
        budget = 0.0
        with nc.named_scope(f"attn{g}"):
            yt_g = [p_yt.tile([128, QG], CDT, tag="yt", name=f"yt{g}_{c}")
                    for c in range(4)]
            yt_gs[g] = yt_g
            for hp in range(4):
                if g == 0 and hp == 1:
                    # group-0 prefetch DMAs deferred past hp0 so their
                    # descriptor generation stays off the startup critical
                    # path on the sync engine.
                    dma_x8(1)
                    dma_x(1, list(range(8)), p_xt)
                    wp_t = p_wp.tile([128, 4, C], CDT, tag="wp")
                    nc.sync.dma_start(out=wp_t, in_=wpt[:, :, :])
                    for kc in range(4):
                        wp_sb.append(wp_t[:, kc, :])
                # the in-order PE queue deadlocks on any backward
                # dependency: this hp's q/k chains must be fully EMITTED
                # before its first S matmul (only group 0 has the
                # intra-group deadline).
                while ((g, hp) not in chains_done
                       or (g, 4 + hp) not in chains_done):
                    fill.pop(0)()
                psy = [ps_y.tile([128, QG], F32, tag="psy",
                                 name=f"psy{g}_{hp}_{r}") for r in range(2)]
                # software pipeline: emit S(kb), fill, AV(kb-1)
                es_q = []   # (kb, es tile)

                def emit_S(kb, hp=hp, g=g):
                    j = kb - 4 * g
                    c0 = max(0, 128 * j)
                    vis = slice(c0, QG)
                    ps = sh_tile(f"pss{g}_{hp}_{kb}")
                    for r in (0, 1):
                        row = slice(64 * r, 64 * r + 64)
                        nc.tensor.matmul(
                            ps[:, r, vis],
                            k_sb[hp][row, kb * 128:(kb + 1) * 128],
                            q_gs[g][hp][row, vis], start=True, stop=True,
                            skip_group_check=True)
                    es = p_es.tile([128, 2, QG], ADT, tag="es")
                    nc.scalar.activation(out=es[:, :, vis], in_=ps[:, :, vis],
                                         func=EXP, scale=SCALE)
                    if j >= 0:
                        # one mask multiply covering BOTH heads (tri
                        # broadcast over the r dim): these ops are
                        # overhead-dominated, so halving the count saves
                        # ~200ns per diagonal block on the DVE queue
                        nc.vector.tensor_mul(
                            es[:, :, c0:c0 + 128], es[:, :, c0:c0 + 128],
                            tri_sb.rearrange("p (o t) -> p o t",
                                             o=1).to_broadcast([KB, 2, KB]))
                    es_q.append((kb, es))

                def emit_AV(hp=hp, g=g, K_g=K_g):
                    kb, es = es_q.pop(0)
                    j = kb - 4 * g
                    c0 = max(0, 128 * j)
                    vis = slice(c0, QG)
                    # the v chain producing vp_sb[kb] must already be
                    # emitted (in-order PE queue): force-drain fill if not
                    while len(vp_sb) <= kb:
                        fill.pop(0)()
                    for r in (0, 1):
                        h = 2 * hp + r
                        nc.tensor.matmul(psy[r][:, vis],
                                         vp_sb[kb][:, h, :],
                                         es[:, r, vis], start=kb == 0,
                                         stop=kb == K_g - 1,
                                         skip_group_check=True)

                # stride-2 software pipeline: [S(b), S(b+1)] [fill]
                # [AV(b-2), AV(b-1)] — S leads AV by two blocks so the AVs
                # never wait on exp, and the pair batching halves the
                # per-insertion PE restart cost.
                for base in range(0, K_g, 2):
                    budget += 2 * rate
                    emit_S(base)
                    emit_S(base + 1)
                    if base == (4 if K_g > 4 else 2) and pending_norm:
                        # flush the PREVIOUS hp's deferred normalize here:
                        # by now its sums input is ~4us old, so the Ln never
                        # suspends the in-order ACT queue waiting on DVE
                        # (which used to block ready attention exps ~2us and
                        # stall the PE on psum-slot reuse).
                        pending_norm.pop(0)()
                    while budget >= 1.0 and fill:
                        fill.pop(0)()
                        budget -= 1.0
                    while len(es_q) > 2:
                        emit_AV()
                while es_q:
                    emit_AV()

                tail = g == NG - 1 and hp == 3
                # normalize off the PE critical path: psum -> sbuf, the two
                # heads' denominators batched into single Ln/Exp ACT ops,
                # DMA broadcast, scale on gpsimd (so the broadcast's DMA
                # roundtrip latency never head-of-line-blocks the DVE
                # queue, which gates chain psum reuse and the tri masks).
                # denominators FIRST, straight from psum row 64 (2 tiny
                # copies): sums is then ready ~3us before the deferred Ln
                # reaches the ACT queue head, so the in-order ACT queue
                # never suspends waiting on it (it used to sit behind the
                # big ysb/fill casts on the DVE queue)
                sums = p_rec.tile([33, QG], F32R, tag="sums",
                                  name=f"sm{g}_{hp}")
                for r in (0, 1):
                    nc.vector.tensor_copy(out=sums[32 * r:32 * r + 1, :],
                                          in_=psy[r][64:65, :])
                ysbs = []
                for r in (0, 1):
                    ysb = p_ysb.tile([65, QG], F32R, tag="ysb",
                                     name=f"ysb{g}_{hp}_{r}")
                    nc.vector.tensor_copy(out=ysb, in_=psy[r][0:65, :])
                    ysbs.append(ysb)
                # reciprocal via Ln + Exp(-x) on ACT: the native DVE
                # InstReciprocal takes ~3.3us per [33,512] tile and
                # head-of-line blocks the DVE queue (masks, chain copies) --
                # measured +25us end-to-end vs this two-op ACT path.
                if tail:
                    lns = p_rec.tile([33, QG], F32, tag="lns",
                                     name=f"ln{g}_{hp}")
                    nc.scalar.activation(out=lns, in_=sums.bitcast(F32),
                                         func=mybir.ActivationFunctionType.Ln)
                    rec = p_rec.tile([33, QG], F32R, tag="rec",
                                     name=f"rec{g}_{hp}")
                    nc.scalar.activation(out=rec, in_=lns, func=EXP,
                                         scale=-1.0)
                    # keep the PE busy (and HAM warm) through the final
                    # normalize chain; emitted after the normalize's DVE/ACT
                    # ops so those queues stay clear of reserve work
                    for ch in tail_reserve:
                        ch()
                    # kernel tail: PE broadcast straight from rec, and the
                    # yt multiply reads the PSUM result directly — the DMA
                    # roundtrip latency would gate proj3
                    psbs = []
                    for r in (0, 1):
                        psb = sh_tile(f"psbx{r}")
                        nc.tensor.matmul(
                            psb[0:64, 0, :],
                            ones_sb[32 * r:32 * r + 1, :],
                            rec[32 * r:32 * r + 1, :],
                            start=True, stop=True, skip_group_check=True)
                        psbs.append(psb)
                    # both tail muls on DVE: gpsimd cannot read PSUM (psb)
                    for r in (0, 1):
                        nc.vector.tensor_mul(yt_g[hp][64 * r:64 * r + 64, :],
                                             ysbs[r][0:64, :].bitcast(F32),
                                             psbs[r][0:64, 0, :])
                else:
                    def norm_b(g=g, hp=hp, sums=sums, ysbs=ysbs,
                               yt_hp=yt_g[hp]):
                        lns = p_rec.tile([33, QG], F32, tag="lns",
                                         name=f"ln{g}_{hp}")
                        nc.scalar.activation(
                            out=lns, in_=sums.bitcast(F32),
                            func=mybir.ActivationFunctionType.Ln)
                        rec = p_rec.tile([33, QG], F32R, tag="rec",
                                         name=f"rec{g}_{hp}")
                        nc.scalar.activation(out=rec, in_=lns, func=EXP,
                                             scale=-1.0)
                        bc = p_bc.tile([64, 2, QG], F32, tag="bc",
                                       name=f"bc{g}_{hp}")
                        recd = p_drb.tile([33, QG], F32, tag="recd",
                                          name=f"recd{g}_{hp}")
                        nc.sync.dma_start(out=recd, in_=rec.bitcast(F32))
                        # single broadcast DMA for both heads:
                        # row 32r -> 64 rows
                        nc.sync.dma_start(
                            out=bc,
                            in_=recd[0:33:32, :].rearrange(
                                "(o j) t -> o j t",
                                o=1).to_broadcast([64, 2, QG]))
                        mul_eng = nc.vector if MUL == "dve" else nc.gpsimd
                        for r in (0, 1):
                            mul_eng.tensor_mul(yt_hp[64 * r:64 * r + 64, :],
                                               ysbs[r][0:64, :].bitcast(F32),
                                               bc[:, r, :])
                    pending_norm.append(norm_b)
        while fill:
            fill.pop(0)()

    with nc.named_scope("proj3"):
        for m in range(8):
            # alternate the psum->sbuf copies across ACT and DVE so the
            # final stores drain without queue backlog
            proj_chain(NG - 1, m, copy_on_act=m % 2 == 0)()


def _build_nc():
    from contextlib import ExitStack
    nc = bass.Bass(trn_type="TRN2")
    # all tensors host-pre-tiled so every DMA has >=2KB contiguous
    # per-partition runs
    xt = nc.dram_tensor("xt", [NG, 128, 8, QG], CDT, kind="ExternalInput")
    x8d = nc.dram_tensor("x8", [NG, 128, 8, QG], FP8, kind="ExternalInput")
    wqkt = nc.dram_tensor("wqkt", [4, 128, 2, CLOC], CDT,
                          kind="ExternalInput")
    wk8d = nc.dram_tensor("wk8", [128, 4, 2, CLOC], FP8,
                          kind="ExternalInput")
    wvt = nc.dram_tensor("wvt", [128, 8, CLOC], CDT, kind="ExternalInput")
    wpt = nc.dram_tensor("wpt", [128, 4, C], CDT, kind="ExternalInput")
    tri = nc.dram_tensor("tri", [KB, KB], CDT, kind="ExternalInput")
    ot = nc.dram_tensor("ot", [NG, 128, 8, QG], CDT, kind="ExternalOutput")
    with tile.TileContext(nc) as tc:
        with ExitStack() as ctx:
            _build_body(nc, tc, ctx, xt, x8d, wqkt, wk8d, wvt, wpt, tri, ot)
    return nc


LAST_RESULTS = None
_NC_CACHE = None


def kernel(x, W_qkv, W_proj):
    global LAST_RESULTS, _NC_CACHE
    x = np.asarray(x, dtype=np.float32)
    W_qkv = np.asarray(W_qkv, dtype=np.float32)
    W_proj = np.asarray(W_proj, dtype=np.float32)

    if _NC_CACHE is None:
        _NC_CACHE = _build_nc()
    nc = _NC_CACHE
    import ml_dtypes
    tri = np.triu(np.ones((KB, KB), np.float32))
    in_maps = []
    for core in range(8):
        b, hg = core // 2, core % 2
        rq = slice(CLOC * hg, CLOC * hg + CLOC)
        Wq = W_qkv[0:C][rq]
        Wk = W_qkv[C:2 * C][rq]
        Wv = W_qkv[2 * C:3 * C][rq]
        # x[b].T is [C, T]: tile to [NG, 128, 8, QG] with
        # xt[g, p, kc, t] = x.T[kc*128+p, g*QG+t]
        xtb = x[b].T.reshape(8, 128, NG, QG).transpose(2, 1, 0, 3)
        wq4 = Wq.T.reshape(4, 2, 128, CLOC).transpose(0, 2, 1, 3)
        # k weights go fp8e4 (DoubleRow): pre-scale by KSC=64 so the
        # uniform(-1/32,1/32) values sit in e4m3's normal range; the 1/64
        # is folded into the exp scale on S
        wk8 = ((Wk.T * KSC).reshape(4, 2, 128, CLOC)
               .transpose(2, 0, 1, 3))       # [128, 4, 2, CLOC]
        wv8 = Wv.T.reshape(8, 128, CLOC).transpose(1, 0, 2)
        wp4 = W_proj[:, rq].T.reshape(4, 128, C).transpose(1, 0, 2)
        _c = lambda a: np.ascontiguousarray(a).astype(ml_dtypes.bfloat16)
        _c8 = lambda a: np.ascontiguousarray(a).astype(ml_dtypes.float8_e4m3)
        in_maps.append({
            "xt": _c(xtb),
            "x8": _c8(xtb),
            "wqkt": _c(wq4),
            "wk8": _c8(wk8),
            "wvt": _c(wv8),
            "wpt": _c(wp4),
            "tri": _c(tri),
        })

    trace = os.environ.get("ATTN_BASS_TRACE") == "1"
    last_exc = None
    out = None
    for attempt in range(4):
        try:
            res = run_bass_kernel_spmd(nc, in_maps, core_ids=list(range(8)),
                                       trace=trace)
        except Exception as e:  # transient NRT device errors happen
            last_exc = e
            import time as _time
            _time.sleep(2.0)
            continue
        LAST_RESULTS = res
        out = np.empty((B, T, C), np.float32)
        for b in range(B):
            acc = (res.results[2 * b]["ot"].astype(np.float32)
                   + res.results[2 * b + 1]["ot"].astype(np.float32))
            # [NG, 128, 8, QG] -> [C, T] -> [T, C]
            out[b] = acc.transpose(2, 1, 0, 3).reshape(C, T).T
        # rare transient device flakes can corrupt a run: retry on any
        # non-finite output
        if np.isfinite(out).all():
            return out
    if out is None:
        raise last_exc
    return out



# revision 58
# speedup vs baseline: 1.0289x; 1.0188x over previous
"""Trainium2 Bass kernel for NanoAttention (B=4, T=2048, C=1024, H=16, causal).

Sharding: 8 cores = 4 batches x 2 head-groups (8 heads each).
Per core (b, hg):
  - column-parallel qkv:  q,k produced in [channel, token] (transposed) layout,
    v produced in [token, channel] (natural) layout with an appended ones
    column per head (fuses the softmax denominator into the AV matmul).
  - causal attention per head with S^T in [key, query] layout; exp on ACT with
    scale=1/sqrt(D); block-causal skipping + triangular masking on the
    diagonal blocks.
  - row-parallel proj producing a partial output [C, T] in bf16; the host adds
    the two head-group partials per batch and transposes back.

Session-2 optimizations (271us -> ~252.5us min / ~253.4us median):
  - group-0 q/k chain copies on ACT + hp1's chains first in the fill
    order: attn0's hp turnover (~3.4us) outruns the congested DVE queue,
    and the next hp's S matmuls gate on those copies (-3us).
  - deferred-normalize queue runs up to 2 hps deep (flush when 2 pending
    or 2 pairs in; forced drain before the tail): a 1-hp deferral was not
    enough in attn0 where hps are only 2 pair-iterations long.
  - k projection in fp8e4 DoubleRow: W_k host-prescaled by 64 (fp8e4
    normal range; 1/64 folded into the exp scale) and x in fp8; each k
    chain is 4 matmuls of 256-contraction instead of 8 of 128 (half the
    PE time; DoubleRow measured at the SAME ns/instr as bf16 = 2x MACs,
    NOT the cost model's 0.5 cyc/row). fp8 on BOTH q and k sims to
    2.0e-2 rel err (at the gate); k-only gives 1.2e-2 measured.
  - one shared 3-slot psum pool (6 banks) for S blocks + fill chains
    (deeper rotation absorbs ACT/DVE latency spikes).
  - per-hp normalize split: denominators copied psum->sbuf FIRST (2 tiny
    copies, so sums is ready early), then Ln/Exp/broadcast/muls deferred
    into the NEXT hp's pair loop -- the in-order ACT queue used to
    suspend ~2-3us at the Ln waiting on DVE-queued sums, blocking ready
    attention exps and stalling the PE on psum-slot reuse.
  - diag tri-masks cover both heads in one DVE op (tri broadcast over r).
  - vp tiles pre-allocated at startup (ones column memset in idle time,
    zero-pad dropped: garbage cols 65:127 feed psum partitions nobody
    reads).
  - the last two output stores are unpaired so the final DMA launches as
    soon as its own chain lands.
  Failed experiments (recorded so they are not retried): DVE
  InstReciprocal = 3.3us/[33,512] (queue-blocking; Ln+Exp on ACT is far
  better, and ACT-table Reciprocal is in a different table set than Exp
  so it would pay 1.3us table switches); dma_start on the ACT HWDGE
  queue corrupts input data in this container; a single static psum tile
  with python-side slot rotation (for paired exps over adjacent slots)
  serializes the kernel to ~490us (subview dependency tracking is
  tile-granular); exp pairing via pool tiles is psum-capacity-blocked;
  hi/lo fp8 splits lose (3 terms x 2x-MACs > bf16); fill rebalancing
  (v(3) into attn2 etc.) only relocates the starvation.

Optimizations vs the 319us v1 baseline (session 1, ~271us, ~15%):
  - software-pipelined emission with a stride-2 rotation: the PE queue is
    in-order, so [S(b), S(b+1)] [dense fill] [AV(b-2), AV(b-1)] keeps the
    AVs off the exp critical path and the pair batching halves the ~130ns
    per-insertion PE restart cost.
  - dense work (qkv of the next group, v chains, ALL proj chains) is
    drip-fed between attention blocks in whole-chain granularity; proj is
    deferred to attn3, which has by far the most ACT(exp)-paced attention
    to fill; 5 chains are reserved for the kernel tail so the PE (and the
    HAM clock gate) stays busy through the final normalize.
  - every dma_start costs ~630ns of serial descriptor generation on the
    sync engine: inputs/outputs are host-pre-tiled ([128, n, free] with
    >=2KB contiguous per-partition runs) and loaded/stored with few
    multi-chunk transfers; output stores are paired.
  - PE pre-warm: ~15 dummy matmuls gated only on a memset lift the HAM
    clock gate from 4/8 (1.2GHz) to 8/8 (2.4GHz) before the first real
    matmul's DMA dependencies can possibly arrive (~13us).
  - vp stationaries padded to 128 columns (v | ones | zeros): full-width
    weights qualify for FWL so AV weight loads hide under the previous
    matmul (65-wide loads cannot and cost ~+120ns per AV pair).
  - the softmax reciprocal broadcast goes SBUF->DRAM->broadcast-DMA, and
    the yt normalize multiplies run on gpsimd so the roundtrip latency
    never head-of-line-blocks the in-order DVE queue (which gates dense
    chain psum reuse and the causal tri masks); the kernel-tail broadcast
    uses a PE ones-matmul directly from rec with the multiply reading
    PSUM, skipping two copies.
  - output DMA in bf16 (host accumulates the two head-group partials in
    f32; rel err unaffected at 4.3e-3 vs the 2e-2 gate).
  (gpsimd partition_broadcast and the custom-DVE reciprocal_approx_fast
  do not compile in this container's walrus: "ISA wrong length".)
"""
import os
import sys

sys.path.insert(0, '/opt/trn_rl_repo')

import numpy as np
import orjson

import concourse.bass as bass
import concourse.mybir as mybir
import concourse.tile as tile
from concourse.bass_utils import run_bass_kernel_spmd

# ---------------------------------------------------------------------------
# Workaround for this container's walrus build: it enforces the HW limit of
# one sync-wait per instruction (two for EventSemaphore), but Tile's sem
# assignment can emit more (kernel-tail Drain waits on every DMA queue used;
# HWDGE stores can pick up two queue waits). Split the overflow onto
# preceding pure-wait EventSemaphore instructions on the same engine at
# JSON-serialization time so every compile path is covered.
# ---------------------------------------------------------------------------


def _split_multi_waits(data):
    n_split = 0
    for func in data.get("functions", []):
        for blk in func.get("blocks", []):
            insts = blk.get("instructions")
            if not insts:
                continue
            out = []
            for inst in insts:
                si = inst.get("sync_info")
                waits = (si or {}).get("on_wait") or []
                cap = 2 if inst.get("opcode") == "EventSemaphore" else 1
                if len(waits) > cap and "engine" in inst:
                    extra = waits[:-cap]
                    si["on_wait"] = waits[-cap:]
                    for i in range(0, len(extra), 2):
                        n_split += 1
                        out.append({
                            "debug": inst.get("debug"),
                            "engine": inst["engine"],
                            "ins": [],
                            "outs": [],
                            "name": f"{inst['name']}_wsplit{n_split}",
                            "opcode": "EventSemaphore",
                            "sync_info": {"on_wait": extra[i:i + 2],
                                          "on_update": []},
                        })
                out.append(inst)
            blk["instructions"] = out
    return data


_orig_to_json_bytes = bass.Bass.to_json_bytes


def _patched_to_json_bytes(self):
    return orjson.dumps(_split_multi_waits(orjson.loads(_orig_to_json_bytes(self))))


bass.Bass.to_json_bytes = _patched_to_json_bytes

# ---------------------------------------------------------------------------

B, T, C = 4, 2048, 1024
N_HEAD, D = 16, 64
HLOC = 8          # heads per core
CLOC = HLOC * D   # 512 local qkv channels per core
QG = 512          # query-group width
NG = T // QG      # 4 query groups
KB = 128          # key-block width
F32R = mybir.dt.float32r
F32 = mybir.dt.float32
BF16 = mybir.dt.bfloat16
CDT = BF16
ADT = CDT
FP8 = mybir.dt.float8e4
DR = mybir.MatmulPerfMode.DoubleRow
KSC = 64.0        # host pre-scale on W_k so fp8e4 stays in normal range
EXP = mybir.ActivationFunctionType.Exp
SCALE = 1.0 / np.sqrt(D) / KSC   # folds the k fp8 pre-scale out of S
# reciprocal broadcast path: "dma" (DRAM roundtrip broadcast) or "gpsimd"
# (partition_broadcast — does NOT compile in this container's walrus:
# "ISA wrong length", same for the custom-DVE reciprocal_approx_fast).
BCAST = os.environ.get("ATTN_BCAST", "dma")
# engine for the yt normalize multiplies: "gpsimd" keeps the DMA-broadcast
# wait out of the DVE queue; "dve" is the fallback.
MUL = os.environ.get("ATTN_MUL", "gpsimd")


def _build_body(nc, tc, ctx, xt, x8d, wqkt, wk8d, wvt, wpt, tri, ot):
    p_wqk = ctx.enter_context(tc.tile_pool(name="wqk", bufs=4))
    p_wk8 = ctx.enter_context(tc.tile_pool(name="wk8", bufs=1))
    p_x8 = ctx.enter_context(tc.tile_pool(name="x8", bufs=2))
    p_wv = ctx.enter_context(tc.tile_pool(name="wv", bufs=1))
    p_wp = ctx.enter_context(tc.tile_pool(name="wp", bufs=1))
    p_xt0 = ctx.enter_context(tc.tile_pool(name="xt0", bufs=4))
    p_xt = ctx.enter_context(tc.tile_pool(name="xt", bufs=2))
    p_k = ctx.enter_context(tc.tile_pool(name="ksb", bufs=4))
    p_q = ctx.enter_context(tc.tile_pool(name="qsb", bufs=8))
    p_vp = ctx.enter_context(tc.tile_pool(name="vp", bufs=16))
    p_es = ctx.enter_context(tc.tile_pool(name="es", bufs=7))
    # all four groups' yt stay alive: proj is deferred to attn3/tail
    p_yt = ctx.enter_context(tc.tile_pool(name="yt", bufs=16))
    p_ost = ctx.enter_context(tc.tile_pool(name="ost", bufs=4))
    p_one = ctx.enter_context(tc.tile_pool(name="one", bufs=1))
    p_rec = ctx.enter_context(tc.tile_pool(name="rec", bufs=3))
    # ysb tiles live until their (up to 2-hp-deferred) normalize muls run:
    # 3 hps x 2 heads alive
    p_ysb = ctx.enter_context(tc.tile_pool(name="ysb", bufs=6))
    p_bc = ctx.enter_context(tc.tile_pool(name="bc", bufs=2))
    if BCAST == "dma":
        p_drb = ctx.enter_context(tc.tile_pool(name="drb", bufs=2, space="DRAM"))
    # ONE shared 3-slot psum pool (6 banks) for S blocks AND dense-fill
    # chains: the extra rotation depth gives the S exp (and fill copies)
    # ~2-3us of slack before their slot is reused, absorbing ACT/DVE queue
    # latency spikes that used to stall the in-order PE queue ~1.4us at a
    # time with the old 2-buf ps_s.
    # NOTE: a single static [128,3,2,QG] psum tile with python-side slot
    # rotation (to enable paired exps over adjacent slots) serializes the
    # whole kernel to ~490us: the tile framework's dependency tracking on
    # subviews of one tile is far more conservative than across pool tiles.
    # Keep pool-based rotation.
    ps_sh = ctx.enter_context(tc.tile_pool(name="pssh", bufs=3, space="PSUM"))
    ps_y = ctx.enter_context(tc.tile_pool(name="psy", bufs=2, space="PSUM"))

    def sh_tile(name):
        return ps_sh.tile([128, 2, QG], F32, tag="pss", name=name)

    # ---- static state ----
    # Each dma_start costs ~630ns of serial descriptor-generation on the
    # sync engine, so inputs are loaded with FEW multi-chunk transfers:
    # contraction-chunk views are AP slices of [128, n, free] tiles.
    wqk_sb = [None] * 8   # views: wqk_sb[kc] = [128, CLOC] (q-only, bf16)
    wv_sb = [None] * 8
    wp_sb = []
    xt_g = [[None] * 8 for _ in range(NG)]
    x8_g = [None] * NG    # [128, 8, QG] fp8 tiles (k-chain moving operand)
    wk8_sb = None         # [128, 4, 2, CLOC] fp8 (k DoubleRow stationaries)
    tri_sb = p_one.tile([KB, KB], CDT, tag="tri")
    ones_sb = p_one.tile([33, 64], F32R, tag="ones")
    nc.vector.memset(ones_sb.bitcast(F32), 1.0)
    k_sb = [p_k.tile([128, T], ADT, tag="ksb", name=f"ksb{c}") for c in range(4)]
    q_gs = [[None] * 4 for _ in range(NG)]
    vp_sb = []        # grows to 16 tiles, 4 per group
    yt_gs = [None] * NG

    def dma_x(g, kcs, pool, eng=None):
        # one dma_start covering contraction chunks kcs (list); the host
        # pre-tiles x as [NG, 128, 8, QG] so each partition's run is
        # contiguous (n*1KB descriptors instead of n*128)
        n = len(kcs)
        t = pool.tile([128, n, QG], CDT, tag="xt", name=f"xt{g}_{kcs[0]}")
        (eng or nc.sync).dma_start(out=t, in_=xt[g, :, kcs[0]:kcs[0] + n, :])
        for i, kc in enumerate(kcs):
            xt_g[g][kc] = t[:, i, :]

    def dma_x8(g):
        t = p_x8.tile([128, 8, QG], FP8, tag="x8", name=f"x8_{g}")
        nc.sync.dma_start(out=t, in_=x8d[g, :, :, :])
        x8_g[g] = t

    def dma_wqk(kc0):
        # one dma_start per PAIR of contraction chunks (host layout
        # [4, 128, 2, CLOC], q weights only -- k weights arrive as fp8)
        t = p_wqk.tile([128, 2, CLOC], CDT, tag="wqk", name=f"wqk{kc0}")
        nc.sync.dma_start(out=t, in_=wqkt[kc0 // 2, :, :, :])
        wqk_sb[kc0] = t[:, 0, :]
        wqk_sb[kc0 + 1] = t[:, 1, :]

    # ---- dense-fill machinery: each fill item emits ONE FULL accumulation
    # chain (8 or 4 back-to-back matmuls + epilogue). Chain granularity
    # keeps the PE instruction stream uniform: scattering single matmuls
    # between attention ops costs a ~130ns restart per insertion. ----
    def qk_chain(g, m):
        # m in 0..7: 0..3 -> q chunks (hp=m), 4..7 -> k chunks (hp=m-4).
        # q chains are bf16 (8 matmuls, 128-contraction each); k chains run
        # fp8e4 DoubleRow (4 matmuls, 256-contraction each -- half the PE
        # time). Only k is quantized: fp8 on both q and k simulates to
        # 2.0e-2 rel err (at the gate), k-only to 1.39e-2.
        def emit(g=g, m=m):
            ps = sh_tile(f"qk{g}_{m}")[:, 0, :]
            # group-0 q/k copies go on ACT: attn0's hp turnover (~3.4us) is
            # faster than the congested DVE queue drains, and the next hp's
            # S matmuls gate on these copies (ACT is light during attn0)
            if m < 4:
                for kc in range(8):
                    nc.tensor.matmul(ps, wqk_sb[kc][:, m * 128:(m + 1) * 128],
                                     xt_g[g][kc], start=kc == 0, stop=kc == 7,
                                     skip_group_check=True)
                qt = p_q.tile([128, QG], ADT, tag="qsb", name=f"q{g}_{m}")
                if g == 0:
                    nc.scalar.activation(
                        out=qt, in_=ps,
                        func=mybir.ActivationFunctionType.Copy)
                else:
                    nc.vector.tensor_copy(out=qt, in_=ps)
                q_gs[g][m] = qt
            else:
                mk = m - 4
                for kcp in range(4):
                    nc.tensor.matmul(
                        ps, wk8_sb[:, kcp, :, mk * 128:(mk + 1) * 128],
                        x8_g[g][:, 2 * kcp:2 * kcp + 2, :],
                        start=kcp == 0, stop=kcp == 3,
                        perf_mode=DR, skip_group_check=True)
                if g == 0:
                    nc.scalar.activation(
                        out=k_sb[mk][:, g * QG:(g + 1) * QG], in_=ps,
                        func=mybir.ActivationFunctionType.Copy)
                else:
                    nc.vector.tensor_copy(
                        out=k_sb[mk][:, g * QG:(g + 1) * QG], in_=ps)
            chains_done.add((g, m))
        return emit

    def v_chain(g, tb):
        def emit(g=g, tb=tb):
            ps = sh_tile(f"v{g}_{tb}")[:, 0, :]
            for kc in range(8):
                nc.tensor.matmul(ps, xt_g[g][kc][:, tb * 128:(tb + 1) * 128],
                                 wv_sb[kc], start=kc == 0, stop=kc == 7,
                                 skip_group_check=True)
            # padded to 128 stationary columns (v | ones | garbage): a full
            # 128-col weight qualifies for FWL, so the AV weight loads hide
            # under the preceding matmul (65-col loads cannot). The ones
            # column was pre-set at startup; cols 65-127 stay uninitialized
            # (their psum partitions 65-127 are never read).
            vp = vp_pre[4 * g + tb]
            nc.vector.tensor_copy(
                out=vp[:, :, 0:64], in_=ps.rearrange("p (h d) -> p h d", d=64))
            vp_sb.append(vp)
        return emit

    ost_pending = {}

    def proj_chain(g, m, copy_on_act=False):
        def emit(g=g, m=m):
            ps = sh_tile(f"pj{g}_{m}")[:, 0, :]
            for c in range(4):
                nc.tensor.matmul(ps, wp_sb[c][:, m * 128:(m + 1) * 128],
                                 yt_gs[g][c], start=c == 0, stop=c == 3,
                                 skip_group_check=True)
            # pair the output stores: one dma_start per two m-chunks
            # (halves the serial descriptor-generation on the sync engine).
            # single_store chains store each chunk on its own so the kernel
            # tail's final store starts as soon as its own chain is done.
            single = g == NG - 1 and m >= 6
            if m % 2 == 0 and not single:
                ost = p_ost.tile([128, 2, QG], CDT, tag="ost",
                                 name=f"ost{g}_{m}")
                ost_pending[g] = ost
            elif single:
                ost = p_ost.tile([128, 2, QG], CDT, tag="ost",
                                 name=f"ost{g}_{m}")
            else:
                ost = ost_pending[g]
            if copy_on_act:
                # keep the in-order DVE queue free for the kernel-tail
                # normalize ops
                nc.scalar.activation(out=ost[:, m % 2, :], in_=ps,
                                     func=mybir.ActivationFunctionType.Copy)
            else:
                nc.vector.tensor_copy(out=ost[:, m % 2, :], in_=ps)
            if single:
                nc.sync.dma_start(out=ot[g, :, m:m + 1, :],
                                  in_=ost[:, m % 2:m % 2 + 1, :])
            elif m % 2 == 1:
                nc.sync.dma_start(out=ot[g, :, m - 1:m + 1, :], in_=ost)
        return emit

    fill = []          # FIFO of pending dense chains
    chains_done = set()

    # ---- PE pre-warm: the HAM clock gate starts at 4/8 (1.2 GHz) and
    # needs ~3.4us of sustained PE busy to release. The first real matmul
    # can't start before ~13us (NEFF init + first DMAs), so burn dummy
    # matmuls that depend only on a memset — the PE is warm before the
    # first chain. ----
    dmy = p_one.tile([1, QG], F32R, tag="dmy")
    nc.vector.memset(dmy.bitcast(F32), 1.0)
    # 24 dummies: the startup DMA stream (4.5MB) only feeds the first real
    # chain at ~18us, and the HAM gate needs sustained PE busy right up to
    # it (15 dummies used to end ~15us and the first chains ran at 1.2GHz)
    for i in range(24):
        psd = sh_tile(f"warm{i}")
        nc.tensor.matmul(psd[0:64, 0, :], ones_sb[0:1, :], dmy,
                         start=True, stop=True, skip_group_check=True)

    # pre-allocate all 16 vp tiles and set their ones column now, while the
    # DVE is idle (startup): the v chains only fill cols 0:64 later, and
    # cols 65:127 stay garbage (their psum partitions are never read).
    vp_pre = []
    for i in range(16):
        vp = p_vp.tile([128, HLOC, 128], ADT, tag="vp", name=f"vp{i}")
        nc.vector.memset(vp[:, :, 64:65], 1.0)
        vp_pre.append(vp)

    # ---- startup: k's fp8 operands first (1MB -> first k chain early),
    # then interleaved wqk/xt0 pair-DMAs ordered by first consumption, wv
    # after (v chains are the first fill); first chains ASAP ----
    wk8_sb = p_wk8.tile([128, 4, 2, CLOC], FP8, tag="wk8")
    nc.sync.dma_start(out=wk8_sb, in_=wk8d[:, :, :, :])
    dma_x8(0)
    for kc0 in range(0, 8, 2):
        dma_wqk(kc0)
        dma_x(0, [kc0, kc0 + 1], p_xt0)
    # wv after the full wqk/xt0 stream: the first q/k chain contracts over
    # ALL kc chunks, while the v chains (fill) only start at the first AVs
    wv_t = p_wv.tile([128, 8, CLOC], CDT, tag="wv")
    nc.sync.dma_start(out=wv_t, in_=wvt[:, :, :])
    for kc in range(8):
        wv_sb[kc] = wv_t[:, kc, :]
    nc.sync.dma_start(out=tri_sb, in_=tri[:, :])

    with nc.named_scope("qkv0"):
        qk_chain(0, 4)()
        qk_chain(0, 0)()

    # ---- main loop over query groups ----
    pending_norm = []   # deferred normalize phase-B closures (max 1 deep)
    for g in range(NG):
        if 0 < g < NG - 1:
            dma_x8(g + 1)
            dma_x(g + 1, list(range(8)), p_xt)

        # fill supply for this group's attention (deadline-ordered):
        #   [g=0 only: v0 + the rest of qkv0], qkv(g+1) q/k + v(g+1).
        # proj has no deadline before the kernel end, so ALL proj chains
        # are deferred to attn3 (the group with by far the most ACT-paced
        # attention to fill) — minus a few reserved for the kernel tail to
        # keep the PE busy (and HAM warm) through the last normalize.
        if g == 0:
            # hp1's q/k chains FIRST so their copies have a full pair of
            # lead time before hp1's S matmuls need them (v0 is pulled
            # forward by emit_AV's force-drain regardless)
            for m in (1, 5):
                fill.append(qk_chain(0, m))
            for tb in range(4):
                fill.append(v_chain(0, tb))
            for m in (2, 6, 3, 7):
                fill.append(qk_chain(0, m))
        if g + 1 < NG:
            for m in (0, 4, 1, 5, 2, 6, 3, 7):
                fill.append(qk_chain(g + 1, m))
            if g + 1 < NG - 1:
                for tb in range(4):
                    fill.append(v_chain(g + 1, tb))
        if g == NG - 1:
            for tb in range(4):
                fill.append(v_chain(g, tb))
            for gp in (0, 1, 2):
                for m in range(8):
                    fill.append(proj_chain(gp, m))
            # reserves keep the PE busy through the final normalize; their
            # psum->sbuf copies go on the ACT engine so the DVE queue
            # stays clear for the normalize chain
            tail_reserve = [proj_chain(2, m, copy_on_act=True)
                            for m in (4, 5, 6, 7)]
            del fill[-4:]

        K_g = 4 * (g + 1)
        # fill rate in CHAINS per attention block (retuned after the fp8 k
        # chains halved: shorter chains -> more pops per pair to keep the
        # exp-paced attention gaps filled)
        rate = [float(v) for v in os.environ.get(
            "ATTN_RATES", "1.45,0.4,0.18,0.42").split(",")][g]
        budget = 0.0
        with nc.named_scope(f"attn{g}"):
            yt_g = [p_yt.tile([128, QG], CDT, tag="yt", name=f"yt{g}_{c}")
                    for c in range(4)]
            yt_gs[g] = yt_g
            for hp in range(4):
                if g == 0 and hp == 1:
                    # group-0 prefetch DMAs deferred past hp0 so their
                    # descriptor generation stays off the startup critical
                    # path on the sync engine.
                    dma_x8(1)
                    dma_x(1, list(range(8)), p_xt)
                    wp_t = p_wp.tile([128, 4, C], CDT, tag="wp")
                    nc.sync.dma_start(out=wp_t, in_=wpt[:, :, :])
                    for kc in range(4):
                        wp_sb.append(wp_t[:, kc, :])
                # the in-order PE queue deadlocks on any backward
                # dependency: this hp's q/k chains must be fully EMITTED
                # before its first S matmul (only group 0 has the
                # intra-group deadline).
                while ((g, hp) not in chains_done
                       or (g, 4 + hp) not in chains_done):
                    fill.pop(0)()
                psy = [ps_y.tile([128, QG], F32, tag="psy",
                                 name=f"psy{g}_{hp}_{r}") for r in range(2)]
                # software pipeline: emit S(kb), fill, AV(kb-1)
                es_q = []   # (kb, es tile)

                def emit_S(kb, hp=hp, g=g):
                    j = kb - 4 * g
                    c0 = max(0, 128 * j)
                    vis = slice(c0, QG)
                    ps = sh_tile(f"pss{g}_{hp}_{kb}")
                    for r in (0, 1):
                        row = slice(64 * r, 64 * r + 64)
                        nc.tensor.matmul(
                            ps[:, r, vis],
                            k_sb[hp][row, kb * 128:(kb + 1) * 128],
                            q_gs[g][hp][row, vis], start=True, stop=True,
                            skip_group_check=True)
                    es = p_es.tile([128, 2, QG], ADT, tag="es")
                    nc.scalar.activation(out=es[:, :, vis], in_=ps[:, :, vis],
                                         func=EXP, scale=SCALE)
                    if j >= 0:
                        # one mask multiply covering BOTH heads (tri
                        # broadcast over the r dim): these ops are
                        # overhead-dominated, so halving the count saves
                        # ~200ns per diagonal block on the DVE queue
                        nc.vector.tensor_mul(
                            es[:, :, c0:c0 + 128], es[:, :, c0:c0 + 128],
                            tri_sb.rearrange("p (o t) -> p o t",
                                             o=1).to_broadcast([KB, 2, KB]))
                    es_q.append((kb, es))

                def emit_AV(hp=hp, g=g, K_g=K_g):
                    kb, es = es_q.pop(0)
                    j = kb - 4 * g
                    c0 = max(0, 128 * j)
                    vis = slice(c0, QG)
                    # the v chain producing vp_sb[kb] must already be
                    # emitted (in-order PE queue): force-drain fill if not
                    while len(vp_sb) <= kb:
                        fill.pop(0)()
                    for r in (0, 1):
                        h = 2 * hp + r
                        nc.tensor.matmul(psy[r][:, vis],
                                         vp_sb[kb][:, h, :],
                                         es[:, r, vis], start=kb == 0,
                                         stop=kb == K_g - 1,
                                         skip_group_check=True)

                # stride-2 software pipeline: [S(b), S(b+1)] [fill]
                # [AV(b-2), AV(b-1)] — S leads AV by two blocks so the AVs
                # never wait on exp, and the pair batching halves the
                # per-insertion PE restart cost.
                norm_flushed = False
                for base in range(0, K_g, 2):
                    budget += 2 * rate
                    emit_S(base)
                    emit_S(base + 1)
                    if (pending_norm and not norm_flushed
                            and (len(pending_norm) >= 2 or base == 4)):
                        # flush a deferred normalize (at most one per hp,
                        # and only once TWO are pending or we are 2 pairs
                        # in): by then its sums input is several us old, so
                        # the Ln never suspends the in-order ACT queue
                        # waiting on DVE (which used to block ready
                        # attention exps and stall the PE on psum reuse --
                        # in attn0 the hp turnover is only ~3.4us, so a
                        # 1-hp deferral was not enough).
                        pending_norm.pop(0)()
                        norm_flushed = True
                    while budget >= 1.0 and fill:
                        fill.pop(0)()
                        budget -= 1.0
                    while len(es_q) > 2:
                        emit_AV()
                while es_q:
                    emit_AV()

                tail = g == NG - 1 and hp == 3
                # normalize off the PE critical path: psum -> sbuf, the two
                # heads' denominators batched into single Ln/Exp ACT ops,
                # DMA broadcast, scale on gpsimd (so the broadcast's DMA
                # roundtrip latency never head-of-line-blocks the DVE
                # queue, which gates chain psum reuse and the tri masks).
                # denominators FIRST, straight from psum row 64 (2 tiny
                # copies): sums is then ready ~3us before the deferred Ln
                # reaches the ACT queue head, so the in-order ACT queue
                # never suspends waiting on it (it used to sit behind the
                # big ysb/fill casts on the DVE queue)
                sums = p_rec.tile([33, QG], F32R, tag="sums",
                                  name=f"sm{g}_{hp}")
                for r in (0, 1):
                    nc.vector.tensor_copy(out=sums[32 * r:32 * r + 1, :],
                                          in_=psy[r][64:65, :])
                ysbs = []
                for r in (0, 1):
                    ysb = p_ysb.tile([65, QG], F32R, tag="ysb",
                                     name=f"ysb{g}_{hp}_{r}")
                    nc.vector.tensor_copy(out=ysb, in_=psy[r][0:65, :])
                    ysbs.append(ysb)
                # reciprocal via Ln + Exp(-x) on ACT: the native DVE
                # InstReciprocal takes ~3.3us per [33,512] tile and
                # head-of-line blocks the DVE queue (masks, chain copies) --
                # measured +25us end-to-end vs this two-op ACT path.
                if tail:
                    # proj3 and the reserves read every hp's yt: drain any
                    # still-deferred normalizes before the tail sequence
                    while pending_norm:
                        pending_norm.pop(0)()
                    lns = p_rec.tile([33, QG], F32, tag="lns",
                                     name=f"ln{g}_{hp}")
                    nc.scalar.activation(out=lns, in_=sums.bitcast(F32),
                                         func=mybir.ActivationFunctionType.Ln)
                    rec = p_rec.tile([33, QG], F32R, tag="rec",
                                     name=f"rec{g}_{hp}")
                    nc.scalar.activation(out=rec, in_=lns, func=EXP,
                                         scale=-1.0)
                    # keep the PE busy (and HAM warm) through the final
                    # normalize chain; emitted after the normalize's DVE/ACT
                    # ops so those queues stay clear of reserve work
                    for ch in tail_reserve:
                        ch()
                    # kernel tail: PE broadcast straight from rec, and the
                    # yt multiply reads the PSUM result directly — the DMA
                    # roundtrip latency would gate proj3
                    psbs = []
                    for r in (0, 1):
                        psb = sh_tile(f"psbx{r}")
                        nc.tensor.matmul(
                            psb[0:64, 0, :],
                            ones_sb[32 * r:32 * r + 1, :],
                            rec[32 * r:32 * r + 1, :],
                            start=True, stop=True, skip_group_check=True)
                        psbs.append(psb)
                    # both tail muls on DVE: gpsimd cannot read PSUM (psb)
                    for r in (0, 1):
                        nc.vector.tensor_mul(yt_g[hp][64 * r:64 * r + 64, :],
                                             ysbs[r][0:64, :].bitcast(F32),
                                             psbs[r][0:64, 0, :])
                else:
                    def norm_b(g=g, hp=hp, sums=sums, ysbs=ysbs,
                               yt_hp=yt_g[hp]):
                        lns = p_rec.tile([33, QG], F32, tag="lns",
                                         name=f"ln{g}_{hp}")
                        nc.scalar.activation(
                            out=lns, in_=sums.bitcast(F32),
                            func=mybir.ActivationFunctionType.Ln)
                        rec = p_rec.tile([33, QG], F32R, tag="rec",
                                         name=f"rec{g}_{hp}")
                        nc.scalar.activation(out=rec, in_=lns, func=EXP,
                                             scale=-1.0)
                        bc = p_bc.tile([64, 2, QG], F32, tag="bc",
                                       name=f"bc{g}_{hp}")
                        recd = p_drb.tile([33, QG], F32, tag="recd",
                                          name=f"recd{g}_{hp}")
                        nc.sync.dma_start(out=recd, in_=rec.bitcast(F32))
                        # single broadcast DMA for both heads:
                        # row 32r -> 64 rows
                        nc.sync.dma_start(
                            out=bc,
                            in_=recd[0:33:32, :].rearrange(
                                "(o j) t -> o j t",
                                o=1).to_broadcast([64, 2, QG]))
                        mul_eng = nc.vector if MUL == "dve" else nc.gpsimd
                        for r in (0, 1):
                            mul_eng.tensor_mul(yt_hp[64 * r:64 * r + 64, :],
                                               ysbs[r][0:64, :].bitcast(F32),
                                               bc[:, r, :])
                    pending_norm.append(norm_b)
        while fill:
            fill.pop(0)()

    with nc.named_scope("proj3"):
        for m in range(8):
            # alternate the psum->sbuf copies across ACT and DVE so the
            # final stores drain without queue backlog
            proj_chain(NG - 1, m, copy_on_act=m % 2 == 0)()


def _build_nc():
    from contextlib import ExitStack
    nc = bass.Bass(trn_type="TRN2")
    # all tensors host-pre-tiled so every DMA has >=2KB contiguous
    # per-partition runs
    xt = nc.dram_tensor("xt", [NG, 128, 8, QG], CDT, kind="ExternalInput")
    x8d = nc.dram_tensor("x8", [NG, 128, 8, QG], FP8, kind="ExternalInput")
    wqkt = nc.dram_tensor("wqkt", [4, 128, 2, CLOC], CDT,
                          kind="ExternalInput")
    wk8d = nc.dram_tensor("wk8", [128, 4, 2, CLOC], FP8,
                          kind="ExternalInput")
    wvt = nc.dram_tensor("wvt", [128, 8, CLOC], CDT, kind="ExternalInput")
    wpt = nc.dram_tensor("wpt", [128, 4, C], CDT, kind="ExternalInput")
    tri = nc.dram_tensor("tri", [KB, KB], CDT, kind="ExternalInput")
    ot = nc.dram_tensor("ot", [NG, 128, 8, QG], CDT, kind="ExternalOutput")
    with tile.TileContext(nc) as tc:
        with ExitStack() as ctx:
            _build_body(nc, tc, ctx, xt, x8d, wqkt, wk8d, wvt, wpt, tri, ot)
    return nc


LAST_RESULTS = None
_NC_CACHE = None


def kernel(x, W_qkv, W_proj):
    global LAST_RESULTS, _NC_CACHE
    x = np.asarray(x, dtype=np.float32)
    W_qkv = np.asarray(W_qkv, dtype=np.float32)
    W_proj = np.asarray(W_proj, dtype=np.float32)

    if _NC_CACHE is None:
        _NC_CACHE = _build_nc()
    nc = _NC_CACHE
    import ml_dtypes
    tri = np.triu(np.ones((KB, KB), np.float32))
    in_maps = []
    for core in range(8):
        b, hg = core // 2, core % 2
        rq = slice(CLOC * hg, CLOC * hg + CLOC)
        Wq = W_qkv[0:C][rq]
        Wk = W_qkv[C:2 * C][rq]
        Wv = W_qkv[2 * C:3 * C][rq]
        # x[b].T is [C, T]: tile to [NG, 128, 8, QG] with
        # xt[g, p, kc, t] = x.T[kc*128+p, g*QG+t]
        xtb = x[b].T.reshape(8, 128, NG, QG).transpose(2, 1, 0, 3)
        wq4 = Wq.T.reshape(4, 2, 128, CLOC).transpose(0, 2, 1, 3)
        # k weights go fp8e4 (DoubleRow): pre-scale by KSC=64 so the
        # uniform(-1/32,1/32) values sit in e4m3's normal range; the 1/64
        # is folded into the exp scale on S
        wk8 = ((Wk.T * KSC).reshape(4, 2, 128, CLOC)
               .transpose(2, 0, 1, 3))       # [128, 4, 2, CLOC]
        wv8 = Wv.T.reshape(8, 128, CLOC).transpose(1, 0, 2)
        wp4 = W_proj[:, rq].T.reshape(4, 128, C).transpose(1, 0, 2)
        _c = lambda a: np.ascontiguousarray(a).astype(ml_dtypes.bfloat16)
        _c8 = lambda a: np.ascontiguousarray(a).astype(ml_dtypes.float8_e4m3)
        in_maps.append({
            "xt": _c(xtb),
            "x8": _c8(xtb),
            "wqkt": _c(wq4),
            "wk8": _c8(wk8),
            "wvt": _c(wv8),
            "wpt": _c(wp4),
            "tri": _c(tri),
        })

    trace = os.environ.get("ATTN_BASS_TRACE") == "1"
    last_exc = None
    out = None
    for attempt in range(4):
        try:
            res = run_bass_kernel_spmd(nc, in_maps, core_ids=list(range(8)),
                                       trace=trace)
        except Exception as e:  # transient NRT device errors happen
            last_exc = e
            import time as _time
            _time.sleep(2.0)
            continue
        LAST_RESULTS = res
        out = np.empty((B, T, C), np.float32)
        for b in range(B):
            acc = (res.results[2 * b]["ot"].astype(np.float32)
                   + res.results[2 * b + 1]["ot"].astype(np.float32))
            # [NG, 128, 8, QG] -> [C, T] -> [T, C]
            out[b] = acc.transpose(2, 1, 0, 3).reshape(C, T).T
        # rare transient device flakes can corrupt a run: retry on any
        # non-finite output
        if np.isfinite(out).all():
            return out
    if out is None:
        raise last_exc
    return out

